# revision 1
# baseline (speedup 1.0000x reference)
"""EnvironmentConsistentAttention on 8 trn2 cores.

Sharding: 4 images x 2 directions (vertical/horizontal neighbor pairs) = 8
independent units, one per core. The horizontal direction of image x equals
the vertical direction of x spatially transposed, so a single SPMD program
handles both: given shifted maps A, B [31,32,256] it returns
(yA, yB) = _corr_recon(A, B), each [31,32,256] (emitted channel-major).

Per-core math (Hp=31, Wp=32, C=256, L=992, k=3):
  pa[(p,q,c), l=(h,w)] = A_pad[h+p, w+q, c]          (zero-padded patches)
  z = pa * pb                                        [2304, L]
  R = z.T @ z                                        [L, L] gram
  att[i,j] = inv[i]*inv[j]*R[i,j];  S = softmax(10*att, axis=j)
  yA = conv_transpose(S, pa) -> ya[l',c] = sum_{p,q,j} S[shift(l',p,q), j]*pa[(p,q,c), j]

att is symmetric pre-softmax, so tiles of R computed as [j-part, i-free] are
directly S.T tiles; exp/softmax-denominator (a cross-partition ones-matmul)
and the reconstruction all run in that transposed layout. S.T is stored in a
[33,34]-padded spatial grid over i so the 9 conv-transpose shifts become pure
access-pattern offsets (zero borders give SAME-padding semantics for free).
Patch norms are folded in as row/column scales of R (host precomputes the
tiny [992] inverse-norm vector).
"""

import numpy as np

Hp, Wp, C = 31, 32, 256
L = Hp * Wp            # 992
PH, PW = Hp + 2, Wp + 2  # 33, 34 padded grid
NPAD = PH * PW         # 1122
KK = 9 * C             # 2304
JC = [(128 * c, 128 if c < 7 else 96) for c in range(8)]   # j/l chunks
HALves = [(0, 512, 0, 16), (512, 480, 16, 15)]  # (i0, n, h0, nh) over i/l'
B_IMG, H_IMG, W_IMG = 4, 32, 32

_CACHE = {}


def _build_program():
    import concourse.bass as bass
    import concourse.tile as tile
    from concourse import bacc, mybir

    f32 = mybir.dt.float32
    f32r = mybir.dt.float32r

    def r(ap):
        return ap.bitcast(f32r)

    nc = bacc.Bacc("TRN2", target_bir_lowering=False, debug=False)

    a_pad = nc.dram_tensor("a_pad", [PH, PW, C], f32, kind="ExternalInput")
    b_pad = nc.dram_tensor("b_pad", [PH, PW, C], f32, kind="ExternalInput")
    a_chw = nc.dram_tensor("a_chw", [C, NPAD], f32, kind="ExternalInput")
    b_chw = nc.dram_tensor("b_chw", [C, NPAD], f32, kind="ExternalInput")
    inv_p = nc.dram_tensor("inv_p", [128, 8], f32, kind="ExternalInput")
    inv_f = nc.dram_tensor("inv_f", [1, L], f32, kind="ExternalInput")
    ya_t = nc.dram_tensor("ya_t", [C, L], f32, kind="ExternalOutput")
    yb_t = nc.dram_tensor("yb_t", [C, L], f32, kind="ExternalOutput")

    with tile.TileContext(nc) as tc:
        from contextlib import ExitStack

        with ExitStack() as ctx:
            const = ctx.enter_context(tc.tile_pool(name="const", bufs=1))
            outp = ctx.enter_context(tc.tile_pool(name="outp", bufs=4))
            tpadp = ctx.enter_context(tc.tile_pool(name="tpad", bufs=8))

            # Constants (input DMAs for these are emitted after the chw
            # loads so the z-build critical path gets the DMA queue first)
            sb_inv_p = const.tile([128, 8], f32, tag="invp")
            sb_inv_b = const.tile([128, L], f32, tag="invb")
            ones_f = const.tile([128, 128], f32, tag="onesf")
            nc.vector.memset(ones_f[:], 1.0)
            ones_k = const.tile([128, 1], f32r, tag="onesk")
            nc.scalar.copy(ones_k[:], ones_f[:, 0:1])
            ones_m = const.tile([1, 128], f32r, tag="onesm")
            nc.scalar.copy(ones_m[:], ones_f[0:1, :])
            from concourse.masks import make_identity

            idn_f = const.tile([128, 128], f32, tag="idnf")
            idn = const.tile([128, 128], f32r, tag="idn")
            make_identity(nc, idn_f[:])
            nc.scalar.copy(idn[:], idn_f[:])
            recip_sb = const.tile([1, L], f32r, tag="recip")
            rb_sb = const.tile([128, L], f32, tag="rbcast")

            # S.T tiles in padded-grid layout, zeroed borders
            tpad = [
                tpadp.tile([128, NPAD], f32r, tag="tpad", name=f"tpad{c}")
                for c in range(8)
            ]

            with ExitStack() as ph1:
                apadp = ph1.enter_context(tc.tile_pool(name="apad", bufs=4))
                zp = ph1.enter_context(tc.tile_pool(name="z", bufs=18))
                psD = ph1.enter_context(
                    tc.tile_pool(name="psD", bufs=1, space="PSUM")
                )

                # Load padded inputs channel-major; build z = pa*pb views
                achw, bchw = [], []
                dma_engs = [nc.sync, nc.scalar, nc.sync, nc.scalar]
                for ch in range(2):
                    ta = apadp.tile([128, NPAD], f32, tag="apad")
                    tb = apadp.tile([128, NPAD], f32, tag="apad")
                    dma_engs[2 * ch].dma_start(
                        out=ta[:], in_=a_chw[128 * ch : 128 * (ch + 1), :]
                    )
                    dma_engs[2 * ch + 1].dma_start(
                        out=tb[:], in_=b_chw[128 * ch : 128 * (ch + 1), :]
                    )
                    achw.append(ta)
                    bchw.append(tb)
                nc.sync.dma_start(out=sb_inv_p[:], in_=inv_p[:, :])
                nc.sync.dma_start(
                    out=sb_inv_b[:], in_=inv_f.ap().to_broadcast([128, L])
                )

                zt = []
                for p in range(3):
                    for q in range(3):
                        for ch in range(2):
                            k = len(zt)
                            zk = zp.tile([128, L], f32r, tag="z")
                            av = achw[ch].rearrange(
                                "c (h w) -> c h w", h=PH, w=PW
                            )[:, p : p + Hp, q : q + Wp]
                            bv = bchw[ch].rearrange(
                                "c (h w) -> c h w", h=PH, w=PW
                            )[:, p : p + Hp, q : q + Wp]
                            nc.vector.tensor_mul(zk[:], av, bv)
                            zt.append(zk)

                # zero S.T borders (gpsimd; only borders matter, interior is
                # overwritten by the exp)
                for c in range(8):
                    tf = tpad[c].bitcast(f32).rearrange(
                        "j (h w) -> j h w", h=PH, w=PW
                    )
                    nc.gpsimd.memset(tf[:, 0:1, :], 0.0)
                    nc.gpsimd.memset(tf[:, PH - 1 : PH, :], 0.0)
                    nc.gpsimd.memset(tf[:, :, 0:1], 0.0)
                    nc.gpsimd.memset(tf[:, :, PW - 1 : PW], 0.0)

                # Gram R = z.T@z per (j-chunk, i-half); scale+exp into tpad;
                # accumulate softmax denominators with ones-matmuls.
                dpsall = psD.tile([1, L], f32, tag="dps", name="dpsall")
                dps = [dpsall[:, i0 : i0 + n] for (i0, n, _, _) in HALves]
                # E is symmetric: compute only i >= 128*jc (upper block
                # triangle incl. diagonal), mirror the rest by PE transpose.
                def ichunks(jc):
                    off = 128 * jc
                    out = []
                    while off < L:
                        n = min(512, L - off)
                        out.append((off, n))
                        off += n
                    return out

                with tc.tile_pool(name="psR", bufs=6, space="PSUM") as psR:
                    for g0, g1 in ((0, 3), (3, 6), (6, 8)):
                        grp = list(enumerate(JC))[g0:g1]
                        rps = {
                            c: [
                                psR.tile(
                                    [128, n], f32, tag="rps", name=f"rps{c}_{ci}"
                                )
                                for ci, (i0, n) in enumerate(ichunks(c))
                            ]
                            for c, _ in grp
                        }
                        # k-major so early matmuls only need early z tiles
                        for k in range(18):
                            for c, (j0, dm) in grp:
                                for ci, (i0, n) in enumerate(ichunks(c)):
                                    nc.tensor.matmul(
                                        rps[c][ci][:dm, :],
                                        zt[k][:, j0 : j0 + dm],
                                        zt[k][:, i0 : i0 + n],
                                        start=(k == 0),
                                        stop=(k == 17),
                                    )
                        for c, (j0, dm) in grp:
                            t3 = tpad[c].rearrange("j (h w) -> j h w", h=PH, w=PW)
                            for ci, (i0, n) in enumerate(ichunks(c)):
                                h0, nh = i0 // Wp, n // Wp
                                itv = t3[:dm, 1 + h0 : 1 + h0 + nh, 1 : 1 + Wp]
                                nc.vector.tensor_mul(
                                    itv,
                                    rps[c][ci][:dm, :],
                                    sb_inv_b[:dm, i0 : i0 + n],
                                )
                                nc.scalar.activation(
                                    itv,
                                    itv,
                                    mybir.ActivationFunctionType.Exp,
                                    scale=sb_inv_p[:dm, c : c + 1],
                                )

                # mirror lower-triangle blocks, then the softmax denominators
                with tc.tile_pool(name="psT", bufs=2, space="PSUM") as psT, \
                        tc.tile_pool(name="tbp", bufs=3) as tbp:
                    for c, (j0, dm) in enumerate(JC):
                        t3j = tpad[c].rearrange("j (h w) -> j h w", h=PH, w=PW)
                        nhj = dm // Wp
                        for ic in range(c):
                            t3s = tpad[ic].rearrange(
                                "j (h w) -> j h w", h=PH, w=PW
                            )
                            srcv = t3s[:128, 1 + 4 * c : 1 + 4 * c + nhj, 1 : 1 + Wp]
                            tbn = tbp.tile(
                                [128, 128], f32r, tag="tbn", name=f"tbn{c}_{ic}"
                            )
                            nc.vector.tensor_copy(tbn[:, :dm], srcv)
                            pst = psT.tile(
                                [128, 128], f32r, tag="pst", name=f"pst{c}_{ic}"
                            )
                            nc.tensor.transpose(pst[:dm, :128], tbn[:, :dm], idn[:, :])
                            nc.vector.tensor_copy(
                                t3j[:dm, 1 + 4 * ic : 1 + 4 * ic + 4, 1 : 1 + Wp],
                                pst[:dm, :128],
                            )
                        for hi, (i0, n, h0, nh) in enumerate(HALves):
                            nc.tensor.matmul(
                                dps[hi],
                                ones_k[:dm, :],
                                t3j[:dm, 1 + h0 : 1 + h0 + nh, 1 : 1 + Wp],
                                start=(c == 0),
                                stop=(c == 7),
                            )

                # 1/denom, broadcast across partitions via K=1 matmul
                rtmp2 = const.tile([1, L], f32, tag="rtmp2")
                nc.vector.reciprocal_approx_fast(out=rtmp2[:, :], in_=dpsall[:, :])
                nc.vector.tensor_copy(recip_sb[:, :], rtmp2[:, :])
                psB = ph1.enter_context(
                    tc.tile_pool(name="psB", bufs=1, space="PSUM")
                )
                bpsall = psB.tile([128, L], f32, tag="bps", name="bpsall")
                for hi, (i0, n, _, _) in enumerate(HALves):
                    nc.tensor.matmul(
                        bpsall[:, i0 : i0 + n],
                        ones_m[:, :],
                        recip_sb[:, i0 : i0 + n],
                        start=True,
                        stop=True,
                    )
                nc.scalar.copy(rb_sb[:, :], bpsall[:, :])

            # Reconstruction, a/b interleaved over one jc sweep; the
            # softmax denominator is applied to each S.T chunk at the top of
            # its jc iteration so recon matmuls chase the scaling.
            # yaT[c, l'] += sum_{p,q,j} paT[j,(p,q,c)]*S.T[j, i(l',p,q)]
            with ExitStack() as ph2:
                patp = ph2.enter_context(tc.tile_pool(name="pat", bufs=4))
                psY = ph2.enter_context(
                    tc.tile_pool(name="psY", bufs=8, space="PSUM")
                )
                yps = [
                    [
                        [
                            psY.tile(
                                [128, n], f32, tag="yps", name=f"yps{t}_{cb}_{hi}"
                            )
                            for hi, (_, n, _, _) in enumerate(HALves)
                        ]
                        for cb in range(2)
                    ]
                    for t in range(2)
                ]
                for c, (j0, dm) in enumerate(JC):
                    h0j, nhj = 4 * c, (4 if c < 7 else 3)
                    t3 = tpad[c].rearrange("j (h w) -> j h w", h=PH, w=PW)
                    for hi, (i0, n, h0, nh) in enumerate(HALves):
                        itv = t3[:dm, 1 + h0 : 1 + h0 + nh, 1 : 1 + Wp]
                        nc.vector.tensor_mul(itv, itv, rb_sb[:dm, i0 : i0 + n])
                    pats = []
                    for t, srcpad in enumerate((a_pad, b_pad)):
                        pt = patp.tile(
                            [128, KK], f32r, tag="pat", name=f"pt{t}_{c}"
                        )
                        for dh in range(nhj):
                            sap = bass.AP(
                                tensor=srcpad.ap().tensor,
                                offset=(h0j + dh) * PW * C,
                                ap=[
                                    [C, Wp],
                                    [PW * C, 3],
                                    [C, 3],
                                    [1, C],
                                ],
                            )
                            nc.sync.dma_start(
                                out=pt[32 * dh : 32 * (dh + 1), :],
                                in_=sap.bitcast(f32r),
                            )
                        pats.append(pt)
                    # last chunk: t-outer so tensor a's accumulators finish
                    # first and their copies/DMA overlap tensor b's matmuls
                    if c < 7:
                        order = [(p, q, t) for p in range(3) for q in range(3) for t in range(2)]
                    else:
                        order = [(p, q, t) for t in range(2) for p in range(3) for q in range(3)]
                    for p, q, t in order:
                        for cb in range(2):
                            lhs = pats[t][
                                :dm,
                                (3 * p + q) * C
                                + 128 * cb : (3 * p + q) * C
                                + 128 * (cb + 1),
                            ]
                            for hi, (i0, n, h0, nh) in enumerate(HALves):
                                rhs = t3[
                                    :dm,
                                    h0 - p + 2 : h0 - p + 2 + nh,
                                    2 - q : 2 - q + Wp,
                                ]
                                nc.tensor.matmul(
                                    yps[t][cb][hi][:, :],
                                    lhs,
                                    rhs,
                                    start=(c == 0 and p == 0 and q == 0),
                                    stop=(c == 7 and p == 2 and q == 2),
                                )

                for t, dram in enumerate((ya_t, yb_t)):
                    for cb in range(2):
                        ysb = outp.tile(
                            [128, L], f32, tag="ysb", name=f"ysb{t}_{cb}"
                        )
                        for hi, (i0, n, _, _) in enumerate(HALves):
                            nc.vector.tensor_copy(
                                ysb[:, i0 : i0 + n], yps[t][cb][hi][:, :]
                            )
                        [nc.sync, nc.scalar, nc.sync, nc.scalar][
                            2 * t + cb
                        ].dma_start(
                            out=dram[128 * cb : 128 * (cb + 1), :], in_=ysb[:]
                        )

    nc.compile()
    return nc


def _get_program():
    if "nc" not in _CACHE:
        _CACHE["nc"] = _build_program()
    return _CACHE["nc"]


def _core_inputs(A, B):
    """A, B: [31,32,256] float32 -> per-core input map."""
    ap = np.zeros((PH, PW, C), np.float32)
    ap[1 : 1 + Hp, 1 : 1 + Wp] = A
    bp = np.zeros((PH, PW, C), np.float32)
    bp[1 : 1 + Hp, 1 : 1 + Wp] = B

    def inv_norm(pad):
        s = (pad.astype(np.float64) ** 2).sum(-1)  # [33,34]
        ss = np.zeros((Hp, Wp))
        for p in range(3):
            for q in range(3):
                ss += s[p : p + Hp, q : q + Wp]
        return 1.0 / np.maximum(np.sqrt(ss), 1e-4)

    inv = (inv_norm(ap) * inv_norm(bp)).reshape(-1)  # [992]
    return {
        "a_pad": ap,
        "b_pad": bp,
        "a_chw": np.ascontiguousarray(ap.transpose(2, 0, 1).reshape(C, NPAD)),
        "b_chw": np.ascontiguousarray(bp.transpose(2, 0, 1).reshape(C, NPAD)),
        "inv_p": np.ascontiguousarray(
            np.pad(10.0 * inv, (0, 1024 - L)).reshape(8, 128).T.astype(np.float32)
        ),
        "inv_f": inv.reshape(1, L).astype(np.float32),
    }


def _untp(y_t):
    # [256, 992] channel-major -> [31, 32, 256]
    return y_t.reshape(C, Hp, Wp).transpose(1, 2, 0)


def kernel(x, mask):
    x = np.asarray(x, dtype=np.float32)
    in_maps = []
    for b in range(B_IMG):
        xb = x[b]
        in_maps.append(_core_inputs(xb[:-1], xb[1:]))
        xt = np.ascontiguousarray(xb.transpose(1, 0, 2))
        in_maps.append(_core_inputs(xt[1:], xt[:-1]))

    from concourse.bass_utils import run_bass_kernel_spmd

    nc = _get_program()
    res = run_bass_kernel_spmd(nc, in_maps, list(range(8))).results

    out = np.empty((B_IMG, H_IMG, W_IMG, C), np.float32)
    for b in range(B_IMG):
        yl = _untp(res[2 * b]["ya_t"])
        yr = _untp(res[2 * b]["yb_t"])
        ylr = np.concatenate(
            [yr[:1], (yr[1:] + yl[:-1]) * 0.5, yl[-1:]], axis=0
        )
        yt = _untp(res[2 * b + 1]["ya_t"]).transpose(1, 0, 2)
        yb = _untp(res[2 * b + 1]["yb_t"]).transpose(1, 0, 2)
        ytb = np.concatenate(
            [yt[:, :1], (yt[:, 1:] + yb[:, :-1]) * 0.5, yb[:, -1:]], axis=1
        )
        out[b] = (ylr + ytb) * 0.5
    return out



# revision 8
# speedup vs baseline: 1.5388x; 1.5388x over previous
"""EnvironmentConsistentAttention on 8 trn2 cores — fp8 DoubleRow version.

Sharding: 4 images x 2 directions (vertical/horizontal neighbor pairs) = 8
independent units, one per core. The horizontal direction of image x equals
the vertical direction of x spatially transposed, so a single SPMD program
handles both: given shifted maps A, B [31,32,256] it returns the fp8
*deviation* reconstruction; the exact uniform-attention part is added on the
host.

Math per core (Hp=31, Wp=32, C=256, L=992, k=3):
  pa[(p,q,c), l] = A_pad[h+p, w+q, c];  z = pa*pb  [2304, L]
  R = z.T @ z;  att = 10*inv_i*inv_j*R;  S = softmax(att, axis=j)
  y = conv_transpose(S, pa)  (and pb)

Key numeric fact for this problem: S is extremely close to uniform (u=1/L),
and y is dominated by the rank-1 uniform term. So split S = u + D and
compute only the deviation term on the accelerator with fp8e4m3 DoubleRow
matmuls (K=256 per pass at 0.5 cycles/row = 4x f32r throughput):
  - gram: z quantized to fp8 on host (scale 4), 9 partition-pair matmul
    groups; exp/softmax-denominator stay f32 (as in the f32r kernel:
    symmetric upper-triangle + PE-transpose mirror; ones-matmul colsums).
  - recon: D = (recip_i*E_ij - u) scaled by 1024, cast to fp8 on the ACT
    engine (Identity activation with bias=-1024/L), patches fp8 from host
    (scale 16). DoubleRow over 4 j-chunk-pairs.
  - uniform term: y_mean[l',c] = u * sum_pq mask(l',p,q) * window_sum(pad)
    computed exactly on host and added back; device output is bf16 (it only
    carries the small deviation term).
Measured model error of this scheme vs the jax reference: l2 ~4.4e-5.
"""

import numpy as np
import ml_dtypes

Hp, Wp, C = 31, 32, 256
L = Hp * Wp            # 992
PH, PW = Hp + 2, Wp + 2  # 33, 34 padded grid
NPAD = PH * PW         # 1122
KK = 9 * C             # 2304
JC = [(128 * c, 128 if c < 7 else 96) for c in range(8)]   # j/l chunks
HALves = [(0, 512, 0, 16), (512, 480, 16, 15)]  # (i0, n, h0, nh) over i/l'
B_IMG, H_IMG, W_IMG = 4, 32, 32

ZSC = 4.0       # host z fp8 scale (per factor; gram R is scaled by ZSC^2)
PSC = 16.0      # host patch fp8 scale
DSC = 1024.0    # device D fp8 scale
E4 = ml_dtypes.float8_e4m3

_CACHE = {}


def _build_program():
    import concourse.bass as bass
    import concourse.tile as tile
    from concourse import bacc, mybir

    f32 = mybir.dt.float32
    f32r = mybir.dt.float32r
    f8 = mybir.dt.float8e4
    bf16 = mybir.dt.bfloat16
    DR = mybir.MatmulPerfMode.DoubleRow

    nc = bacc.Bacc("TRN2", target_bir_lowering=False, debug=False)

    z8d = nc.dram_tensor("z8", [1152, 2 * L], f8, kind="ExternalInput")
    a8d = nc.dram_tensor("a8", [PH, PW, C], f8, kind="ExternalInput")
    b8d = nc.dram_tensor("b8", [PH, PW, C], f8, kind="ExternalInput")
    inv_p = nc.dram_tensor("inv_p", [128, 8], f32, kind="ExternalInput")
    inv_f = nc.dram_tensor("inv_f", [1, L], f32, kind="ExternalInput")
    ya_t = nc.dram_tensor("ya_t", [C, L], bf16, kind="ExternalOutput")
    yb_t = nc.dram_tensor("yb_t", [C, L], bf16, kind="ExternalOutput")

    with tile.TileContext(nc) as tc:
        from contextlib import ExitStack

        with ExitStack() as ctx:
            const = ctx.enter_context(tc.tile_pool(name="const", bufs=1))
            outp = ctx.enter_context(tc.tile_pool(name="outp", bufs=4))
            tpadp = ctx.enter_context(tc.tile_pool(name="tpad", bufs=8))
            z8p = ctx.enter_context(tc.tile_pool(name="z8p", bufs=9))
            patp = ctx.enter_context(tc.tile_pool(name="pat", bufs=8))
            s8p = ctx.enter_context(tc.tile_pool(name="s8p", bufs=4))

            dma_engs = [nc.sync, nc.scalar, nc.gpsimd]

            # z8 pair tiles first: the gram chases these
            z8 = []
            for k in range(9):
                zt = z8p.tile([128, 2, L], f8, tag="z8", name=f"z8_{k}")
                dma_engs[k % 3].dma_start(
                    out=zt[:], in_=z8d[128 * k : 128 * (k + 1), :]
                )
                z8.append(zt)

            # Constants
            sb_inv_p = const.tile([128, 8], f32, tag="invp")
            sb_inv_b = const.tile([128, L], f32, tag="invb")
            nc.sync.dma_start(out=sb_inv_p[:], in_=inv_p[:, :])
            nc.sync.dma_start(
                out=sb_inv_b[:], in_=inv_f.ap().to_broadcast([128, L])
            )
            ones_f = const.tile([128, 128], f32, tag="onesf")
            nc.vector.memset(ones_f[:], 1.0)
            ones_k = const.tile([128, 1], f32r, tag="onesk")
            nc.scalar.copy(ones_k[:], ones_f[:, 0:1])
            sc_f = const.tile([1, 128], f32, tag="scf")
            nc.vector.memset(sc_f[:], DSC)
            sc_m = const.tile([1, 128], f32r, tag="scm")
            nc.scalar.copy(sc_m[:], sc_f[:, :])
            bias_u = const.tile([128, 1], f32, tag="biasu")
            nc.vector.memset(bias_u[:], -DSC / L)
            from concourse.masks import make_identity

            idn_f = const.tile([128, 128], f32, tag="idnf")
            idn = const.tile([128, 128], f32r, tag="idn")
            make_identity(nc, idn_f[:])
            nc.scalar.copy(idn[:], idn_f[:])
            recip_sb = const.tile([1, L], f32r, tag="recip")
            rb_sb = const.tile([128, L], f32, tag="rbcast")

            # Patch-gather tiles (fp8): [j-part, pair-member, (p,q,c_out)].
            # Prefetched for all 4 pairs during the gram phase.
            pt8 = {}
            for t, srcpad in enumerate((a8d, b8d)):
                for P in range(4):
                    pt = patp.tile(
                        [128, 2, KK], f8, tag="pat", name=f"pt{t}_{P}"
                    )
                    for m in range(2):
                        c = 2 * P + m
                        nhj = 4 if c < 7 else 3
                        for dh in range(nhj):
                            sap = bass.AP(
                                tensor=srcpad.ap().tensor,
                                offset=(4 * c + dh) * PW * C,
                                ap=[
                                    [C, Wp],
                                    [PW * C, 3],
                                    [C, 3],
                                    [1, C],
                                ],
                            )
                            dma_engs[(2 * P + m + dh) % 3].dma_start(
                                out=pt[32 * dh : 32 * (dh + 1), m, :],
                                in_=sap,
                            )
                    if P == 3:
                        # chunk 7 has only 3 dh rows: zero the tail rows
                        nc.gpsimd.memset(pt[96:128, 1, :], 0.0)
                    pt8[(t, P)] = pt

            # S.T deviation tiles (fp8, padded grid + 2 overrun cols) per
            # j-chunk pair. Recon reads them as flat [j, m, nh*34] streams.
            s8 = [
                s8p.tile([128, 2, NPAD + 2], f8, tag="s8", name=f"s8_{P}")
                for P in range(4)
            ]
            for P in range(4):
                t4 = s8[P][:, :, :NPAD].rearrange(
                    "j m (h w) -> j m h w", h=PH, w=PW
                )
                for m in range(2):
                    nc.gpsimd.memset(t4[:, m, 0:1, :], 0.0)
                    nc.gpsimd.memset(t4[:, m, PH - 1 : PH, :], 0.0)
                    nc.gpsimd.memset(t4[:, m, :, 0:1], 0.0)
                    nc.gpsimd.memset(t4[:, m, :, PW - 1 : PW], 0.0)
                nc.gpsimd.memset(s8[P][:, :, NPAD : NPAD + 2], 0.0)
            # chunk 7 has dm=96: zero its tail partitions once
            nc.gpsimd.memset(s8[3][96:128, 1, :], 0.0)

            # E tiles in padded-grid layout (f32r), zeroed borders
            tpad = [
                tpadp.tile([128, NPAD], f32r, tag="tpad", name=f"tpad{c}")
                for c in range(8)
            ]
            for c in range(8):
                tf = tpad[c].bitcast(f32).rearrange(
                    "j (h w) -> j h w", h=PH, w=PW
                )
                nc.gpsimd.memset(tf[:, 0:1, :], 0.0)
                nc.gpsimd.memset(tf[:, PH - 1 : PH, :], 0.0)
                nc.gpsimd.memset(tf[:, :, 0:1], 0.0)
                nc.gpsimd.memset(tf[:, :, PW - 1 : PW], 0.0)

            with ExitStack() as ph1:
                psD = ph1.enter_context(
                    tc.tile_pool(name="psD", bufs=1, space="PSUM")
                )
                dpsall = psD.tile([1, L], f32, tag="dps", name="dpsall")
                dps = [dpsall[:, i0 : i0 + n] for (i0, n, _, _) in HALves]

                # Gram R = z.T@z per (j-chunk, i-chunk); E symmetric so only
                # i >= 128*jc is computed, rest mirrored by PE transpose.
                def ichunks(jc):
                    off = 128 * jc
                    out = []
                    while off < L:
                        n = min(512, L - off)
                        out.append((off, n))
                        off += n
                    return out

                with tc.tile_pool(name="psR", bufs=6, space="PSUM") as psR:
                    for g0, g1 in ((0, 3), (3, 6), (6, 8)):
                        grp = list(enumerate(JC))[g0:g1]
                        rps = {
                            c: [
                                psR.tile(
                                    [128, n], f32, tag="rps", name=f"rps{c}_{ci}"
                                )
                                for ci, (i0, n) in enumerate(ichunks(c))
                            ]
                            for c, _ in grp
                        }
                        # k-pair-major so early matmuls only need early z8
                        for k in range(9):
                            for c, (j0, dm) in grp:
                                for ci, (i0, n) in enumerate(ichunks(c)):
                                    nc.tensor.matmul(
                                        rps[c][ci][:dm, :],
                                        z8[k][:, :, j0 : j0 + dm],
                                        z8[k][:, :, i0 : i0 + n],
                                        start=(k == 0),
                                        stop=(k == 8),
                                        perf_mode=DR,
                                    )
                        for c, (j0, dm) in grp:
                            t3 = tpad[c].rearrange("j (h w) -> j h w", h=PH, w=PW)
                            for ci, (i0, n) in enumerate(ichunks(c)):
                                h0, nh = i0 // Wp, n // Wp
                                itv = t3[:dm, 1 + h0 : 1 + h0 + nh, 1 : 1 + Wp]
                                nc.vector.tensor_mul(
                                    itv,
                                    rps[c][ci][:dm, :],
                                    sb_inv_b[:dm, i0 : i0 + n],
                                )
                                nc.scalar.activation(
                                    itv,
                                    itv,
                                    mybir.ActivationFunctionType.Exp,
                                    scale=sb_inv_p[:dm, c : c + 1],
                                )

                # mirror lower-triangle blocks, then the softmax denominators
                with tc.tile_pool(name="psT", bufs=2, space="PSUM") as psT, \
                        tc.tile_pool(name="tbp", bufs=3) as tbp:
                    for c, (j0, dm) in enumerate(JC):
                        t3j = tpad[c].rearrange("j (h w) -> j h w", h=PH, w=PW)
                        nhj = dm // Wp
                        for ic in range(c):
                            t3s = tpad[ic].rearrange(
                                "j (h w) -> j h w", h=PH, w=PW
                            )
                            srcv = t3s[:128, 1 + 4 * c : 1 + 4 * c + nhj, 1 : 1 + Wp]
                            tbn = tbp.tile(
                                [128, 128], f32r, tag="tbn", name=f"tbn{c}_{ic}"
                            )
                            nc.vector.tensor_copy(tbn[:, :dm], srcv)
                            pst = psT.tile(
                                [128, 128], f32r, tag="pst", name=f"pst{c}_{ic}"
                            )
                            nc.tensor.transpose(pst[:dm, :128], tbn[:, :dm], idn[:, :])
                            nc.vector.tensor_copy(
                                t3j[:dm, 1 + 4 * ic : 1 + 4 * ic + 4, 1 : 1 + Wp],
                                pst[:dm, :128],
                            )
                        for hi, (i0, n, h0, nh) in enumerate(HALves):
                            nc.tensor.matmul(
                                dps[hi],
                                ones_k[:dm, :],
                                t3j[:dm, 1 + h0 : 1 + h0 + nh, 1 : 1 + Wp],
                                start=(c == 0),
                                stop=(c == 7),
                            )

                # 1/denom, broadcast (x DSC) across partitions via K=1 matmul
                rtmp2 = const.tile([1, L], f32, tag="rtmp2")
                nc.vector.reciprocal_approx_fast(out=rtmp2[:, :], in_=dpsall[:, :])
                nc.vector.tensor_copy(recip_sb[:, :], rtmp2[:, :])
                psB = ph1.enter_context(
                    tc.tile_pool(name="psB", bufs=1, space="PSUM")
                )
                bpsall = psB.tile([128, L], f32, tag="bps", name="bpsall")
                for hi, (i0, n, _, _) in enumerate(HALves):
                    nc.tensor.matmul(
                        bpsall[:, i0 : i0 + n],
                        sc_m[:, :],
                        recip_sb[:, i0 : i0 + n],
                        start=True,
                        stop=True,
                    )
                nc.scalar.copy(rb_sb[:, :], bpsall[:, :])

            # Reconstruction of the deviation term, DoubleRow over pairs.
            # Outputs accumulate in grid-shaped PSUM tiles (rows of 34 incl
            # 2 junk cols) so each (p,q)-shifted rhs is a single contiguous
            # [j, 2, nh*34] stream (the DoubleRow-compatible 3D form). Two
            # sequential sweeps (tensor a then b), 6 PSUM banks each; sweep
            # a's output copy/DMA overlaps sweep b's matmuls.
            # Per chunk: normalize E by DSC*recip_i (DVE), then cast to fp8
            # with the uniform offset folded into the activation bias.
            SECS = [(0, 11), (11, 10), (21, 10)]  # (h0, nh) over l' rows
            with ExitStack() as ph2:
                psY = ph2.enter_context(
                    tc.tile_pool(name="psY", bufs=6, space="PSUM")
                )
                for t, dram in enumerate((ya_t, yb_t)):
                    yps = [
                        [
                            psY.tile(
                                [128, nh * PW],
                                f32,
                                tag="yps",
                                name=f"yps{t}_{cb}_{si}",
                            )
                            for si, (h0, nh) in enumerate(SECS)
                        ]
                        for cb in range(2)
                    ]
                    for P in range(4):
                        if t == 0:
                            s4 = s8[P][:, :, :NPAD].rearrange(
                                "j m (h w) -> j m h w", h=PH, w=PW
                            )
                            for m in range(2):
                                c = 2 * P + m
                                j0, dm = JC[c]
                                t3 = tpad[c].rearrange(
                                    "j (h w) -> j h w", h=PH, w=PW
                                )
                                for hi, (i0, n, h0, nh) in enumerate(HALves):
                                    itv = t3[
                                        :dm, 1 + h0 : 1 + h0 + nh, 1 : 1 + Wp
                                    ]
                                    nc.vector.tensor_mul(
                                        itv, itv, rb_sb[:dm, i0 : i0 + n]
                                    )
                                    nc.scalar.activation(
                                        s4[:dm, m, 1 + h0 : 1 + h0 + nh, 1 : 1 + Wp],
                                        itv,
                                        mybir.ActivationFunctionType.Identity,
                                        bias=bias_u[:dm, :],
                                    )
                        for p in range(3):
                            for q in range(3):
                                for cb in range(2):
                                    lhs = pt8[(t, P)][
                                        :,
                                        :,
                                        (3 * p + q) * C
                                        + 128 * cb : (3 * p + q) * C
                                        + 128 * (cb + 1),
                                    ]
                                    for si, (h0, nh) in enumerate(SECS):
                                        g0 = (h0 - p + 2) * PW + (2 - q)
                                        nc.tensor.matmul(
                                            yps[cb][si][:, :],
                                            lhs,
                                            s8[P][:, :, g0 : g0 + nh * PW],
                                            start=(P == 0 and p == 0 and q == 0),
                                            stop=(P == 3 and p == 2 and q == 2),
                                            perf_mode=DR,
                                        )
                    for cb in range(2):
                        ysb = outp.tile(
                            [128, L], bf16, tag="ysb", name=f"ysb{t}_{cb}"
                        )
                        for si, (h0, nh) in enumerate(SECS):
                            ypv = yps[cb][si].rearrange(
                                "c (h w) -> c h w", h=nh, w=PW
                            )[:, :, 0:Wp]
                            ysv = ysb[:, h0 * Wp : (h0 + nh) * Wp].rearrange(
                                "c (h w) -> c h w", h=nh, w=Wp
                            )
                            nc.vector.tensor_copy(ysv, ypv)
                        [nc.sync, nc.scalar, nc.sync, nc.scalar][
                            2 * t + cb
                        ].dma_start(
                            out=dram[128 * cb : 128 * (cb + 1), :], in_=ysb[:]
                        )

    nc.compile()
    return nc


def _get_program():
    if "nc" not in _CACHE:
        _CACHE["nc"] = _build_program()
    return _CACHE["nc"]


def _prep_core(A, B):
    """A, B: [31,32,256] float32 -> (input map, host uniform term [L, C])."""
    ap = np.zeros((PH, PW, C), np.float32)
    ap[1 : 1 + Hp, 1 : 1 + Wp] = A
    bp = np.zeros((PH, PW, C), np.float32)
    bp[1 : 1 + Hp, 1 : 1 + Wp] = B

    # patches [3,3,C,L] without materializing: strided windows
    def win(pad, p, q):
        return pad[p : p + Hp, q : q + Wp]  # [Hp, Wp, C]

    ss_a = np.zeros((Hp, Wp))
    ss_b = np.zeros((Hp, Wp))
    z8 = np.empty((1152, 2 * L), dtype=E4)
    zrow = np.empty((C, L), np.float32)
    for p in range(3):
        for q in range(3):
            wa = win(ap, p, q).astype(np.float64)
            wb = win(bp, p, q).astype(np.float64)
            ss_a += (wa * wa).sum(-1)
            ss_b += (wb * wb).sum(-1)
            np.multiply(
                win(ap, p, q).reshape(L, C).T,
                win(bp, p, q).reshape(L, C).T,
                out=zrow,
            )
            kk = 2 * (3 * p + q)  # two 128-row slices per (p,q)
            for half in range(2):
                rows = zrow[128 * half : 128 * (half + 1)]
                pair, mm = divmod(kk + half, 2)
                z8[128 * pair : 128 * (pair + 1), mm * L : (mm + 1) * L] = (
                    ZSC * rows
                ).astype(E4)
    inv = (
        1.0
        / np.maximum(np.sqrt(ss_a), 1e-4)
        / np.maximum(np.sqrt(ss_b), 1e-4)
    ).reshape(-1)

    # host uniform term: y_mean[l', c] = u * sum_pq mask * window-sum
    u = 1.0 / L
    Ug = np.zeros((PH, PW))
    Ug[1 : 1 + Hp, 1 : 1 + Wp] = 1.0
    ymean_a = np.zeros((L, C))
    ymean_b = np.zeros((L, C))
    for p in range(3):
        for q in range(3):
            w = Ug[2 - p : 2 - p + Hp, 2 - q : 2 - q + Wp].reshape(L, 1)
            ymean_a += u * w @ win(ap, p, q).astype(np.float64).sum((0, 1))[None, :]
            ymean_b += u * w @ win(bp, p, q).astype(np.float64).sum((0, 1))[None, :]

    inp = {
        "z8": z8,
        "a8": (PSC * ap).astype(E4),
        "b8": (PSC * bp).astype(E4),
        "inv_p": np.ascontiguousarray(
            np.pad(10.0 * inv, (0, 1024 - L)).reshape(8, 128).T.astype(np.float32)
        ),
        "inv_f": (inv / (ZSC * ZSC)).reshape(1, L).astype(np.float32),
    }
    return inp, ymean_a, ymean_b


def _assemble(res, ymean_a, ymean_b):
    """Device bf16 deviation outputs [C, L] -> full [Hp, Wp, C] pair."""
    sc = 1.0 / (DSC * PSC)
    ya = ymean_a + sc * res["ya_t"].astype(np.float64).T
    yb = ymean_b + sc * res["yb_t"].astype(np.float64).T
    return (
        ya.reshape(Hp, Wp, C).astype(np.float32),
        yb.reshape(Hp, Wp, C).astype(np.float32),
    )


def kernel(x, mask):
    x = np.asarray(x, dtype=np.float32)
    in_maps = []
    hosts = []
    for b in range(B_IMG):
        xb = x[b]
        im, ha, hb = _prep_core(xb[:-1], xb[1:])
        in_maps.append(im)
        hosts.append((ha, hb))
        xt = np.ascontiguousarray(xb.transpose(1, 0, 2))
        im, ha, hb = _prep_core(xt[1:], xt[:-1])
        in_maps.append(im)
        hosts.append((ha, hb))

    from concourse.bass_utils import run_bass_kernel_spmd

    nc = _get_program()
    res = run_bass_kernel_spmd(nc, in_maps, list(range(8))).results

    out = np.empty((B_IMG, H_IMG, W_IMG, C), np.float32)
    for b in range(B_IMG):
        yl, yr = _assemble(res[2 * b], *hosts[2 * b])
        ylr = np.concatenate(
            [yr[:1], (yr[1:] + yl[:-1]) * 0.5, yl[-1:]], axis=0
        )
        yt, yb = _assemble(res[2 * b + 1], *hosts[2 * b + 1])
        yt = yt.transpose(1, 0, 2)
        yb = yb.transpose(1, 0, 2)
        ytb = np.concatenate(
            [yt[:, :1], (yt[:, 1:] + yb[:, :-1]) * 0.5, yb[:, -1:]], axis=1
        )
        out[b] = (ylr + ytb) * 0.5
    return out


# revision 15
# speedup vs baseline: 1.5459x; 1.0046x over previous
"""EnvironmentConsistentAttention on 8 trn2 cores — fp8 DoubleRow version.

Sharding: 4 images x 2 directions (vertical/horizontal neighbor pairs) = 8
independent units, one per core. The horizontal direction of image x equals
the vertical direction of x spatially transposed, so a single SPMD program
handles both: given shifted maps A, B [31,32,256] it returns the fp8
*deviation* reconstruction; the exact uniform-attention part is added on the
host.

Math per core (Hp=31, Wp=32, C=256, L=992, k=3):
  pa[(p,q,c), l] = A_pad[h+p, w+q, c];  z = pa*pb  [2304, L]
  R = z.T @ z;  att = 10*inv_i*inv_j*R;  S = softmax(att, axis=j)
  y = conv_transpose(S, pa)  (and pb)

Key numeric fact for this problem: S is extremely close to uniform (u=1/L),
and y is dominated by the rank-1 uniform term. So split S = u + D and
compute only the deviation term on the accelerator with fp8e4m3 DoubleRow
matmuls (K=256 per pass at 0.5 cycles/row = 4x f32r throughput):
  - gram: z quantized to fp8 on host (scale 4), 9 partition-pair matmul
    groups; exp/softmax-denominator stay f32 (as in the f32r kernel:
    symmetric upper-triangle + PE-transpose mirror; ones-matmul colsums).
  - recon: D = (recip_i*E_ij - u) scaled by 1024, cast to fp8 on the ACT
    engine (Identity activation with bias=-1024/L), patches fp8 from host
    (scale 16). DoubleRow over 4 j-chunk-pairs.
  - uniform term: y_mean[l',c] = u * sum_pq mask(l',p,q) * window_sum(pad)
    computed exactly on host and added back; device output is bf16 (it only
    carries the small deviation term).
Measured model error of this scheme vs the jax reference: l2 ~4.4e-5.
"""

import numpy as np
import ml_dtypes

Hp, Wp, C = 31, 32, 256
L = Hp * Wp            # 992
PH, PW = Hp + 2, Wp + 2  # 33, 34 padded grid
NPAD = PH * PW         # 1122
KK = 9 * C             # 2304
JC = [(128 * c, 128 if c < 7 else 96) for c in range(8)]   # j/l chunks
HALves = [(0, 512, 0, 16), (512, 480, 16, 15)]  # (i0, n, h0, nh) over i/l'
B_IMG, H_IMG, W_IMG = 4, 32, 32

ZSC = 4.0       # host z fp8 scale (per factor; gram R is scaled by ZSC^2)
PSC = 16.0      # host patch fp8 scale
DSC = 1024.0    # device D fp8 scale
E4 = ml_dtypes.float8_e4m3

_CACHE = {}


def _build_program():
    import concourse.bass as bass
    import concourse.tile as tile
    from concourse import bacc, mybir

    f32 = mybir.dt.float32
    f32r = mybir.dt.float32r
    f8 = mybir.dt.float8e4
    bf16 = mybir.dt.bfloat16
    DR = mybir.MatmulPerfMode.DoubleRow

    nc = bacc.Bacc("TRN2", target_bir_lowering=False, debug=False)

    z8d = nc.dram_tensor("z8", [1152, 2 * L], f8, kind="ExternalInput")
    a8d = nc.dram_tensor("a8", [PH, PW, C], f8, kind="ExternalInput")
    b8d = nc.dram_tensor("b8", [PH, PW, C], f8, kind="ExternalInput")
    inv_p = nc.dram_tensor("inv_p", [128, 8], f32, kind="ExternalInput")
    inv_f = nc.dram_tensor("inv_f", [1, L], f32, kind="ExternalInput")
    ya_t = nc.dram_tensor("ya_t", [C, L], bf16, kind="ExternalOutput")
    yb_t = nc.dram_tensor("yb_t", [C, L], bf16, kind="ExternalOutput")

    with tile.TileContext(nc) as tc:
        from contextlib import ExitStack

        with ExitStack() as ctx:
            const = ctx.enter_context(tc.tile_pool(name="const", bufs=1))
            outp = ctx.enter_context(tc.tile_pool(name="outp", bufs=4))
            tpadp = ctx.enter_context(tc.tile_pool(name="tpad", bufs=8))
            z8p = ctx.enter_context(tc.tile_pool(name="z8p", bufs=9))
            patp = ctx.enter_context(tc.tile_pool(name="pat", bufs=8))
            s8p = ctx.enter_context(tc.tile_pool(name="s8p", bufs=4))

            # gpsimd dma_start goes through a slow DIRECT2D path and clogs
            # the gpsimd sequencer (whose memsets gate the exp/cast chain):
            # only sync and scalar issue DMAs.
            dma_engs = [nc.sync, nc.scalar]

            # z8 pair tiles first: the gram chases these
            z8 = []
            for k in range(9):
                zt = z8p.tile([128, 2, L], f8, tag="z8", name=f"z8_{k}")
                dma_engs[k % 2].dma_start(
                    out=zt[:], in_=z8d[128 * k : 128 * (k + 1), :]
                )
                z8.append(zt)

            # Constants
            sb_inv_p = const.tile([128, 8], f32, tag="invp")
            sb_inv_b = const.tile([128, L], f32, tag="invb")
            nc.sync.dma_start(out=sb_inv_p[:], in_=inv_p[:, :])
            nc.sync.dma_start(
                out=sb_inv_b[:], in_=inv_f.ap().to_broadcast([128, L])
            )
            ones_f = const.tile([128, 128], f32, tag="onesf")
            nc.vector.memset(ones_f[:], 1.0)
            ones_k = const.tile([128, 1], f32r, tag="onesk")
            nc.scalar.copy(ones_k[:], ones_f[:, 0:1])
            sc_f = const.tile([1, 128], f32, tag="scf")
            nc.vector.memset(sc_f[:], DSC)
            sc_m = const.tile([1, 128], f32r, tag="scm")
            nc.scalar.copy(sc_m[:], sc_f[:, :])
            bias_u = const.tile([128, 1], f32, tag="biasu")
            nc.vector.memset(bias_u[:], -DSC / L)
            from concourse.masks import make_identity

            idn_f = const.tile([128, 128], f32, tag="idnf")
            idn = const.tile([128, 128], f32r, tag="idn")
            make_identity(nc, idn_f[:])
            nc.scalar.copy(idn[:], idn_f[:])
            recip_sb = const.tile([1, L], f32r, tag="recip")

            # S.T deviation tiles (fp8, padded grid + 2 overrun cols) per
            # j-chunk pair. Recon reads them as flat [j, m, nh*34] streams.
            s8 = [
                s8p.tile([128, 2, NPAD + 2], f8, tag="s8", name=f"s8_{P}")
                for P in range(4)
            ]
            for P in range(4):
                t4 = s8[P][:, :, :NPAD].rearrange(
                    "j m (h w) -> j m h w", h=PH, w=PW
                )
                for m in range(2):
                    nc.gpsimd.memset(t4[:, m, 0:1, :], 0.0)
                    nc.gpsimd.memset(t4[:, m, PH - 1 : PH, :], 0.0)
                    nc.gpsimd.memset(t4[:, m, :, 0:1], 0.0)
                    nc.gpsimd.memset(t4[:, m, :, PW - 1 : PW], 0.0)
                nc.gpsimd.memset(s8[P][:, :, NPAD : NPAD + 2], 0.0)
            # chunk 7 has dm=96: zero its tail partitions once
            nc.gpsimd.memset(s8[3][96:128, 1, :], 0.0)

            # E tiles in padded-grid layout (f32r), zeroed borders
            tpad = [
                tpadp.tile([128, NPAD], f32r, tag="tpad", name=f"tpad{c}")
                for c in range(8)
            ]
            for c in range(8):
                tf = tpad[c].bitcast(f32).rearrange(
                    "j (h w) -> j h w", h=PH, w=PW
                )
                nc.gpsimd.memset(tf[:, 0:1, :], 0.0)
                nc.gpsimd.memset(tf[:, PH - 1 : PH, :], 0.0)
                nc.gpsimd.memset(tf[:, :, 0:1], 0.0)
                nc.gpsimd.memset(tf[:, :, PW - 1 : PW], 0.0)

            # Patch-gather tiles (fp8): [j-part, pair-member, (p,q,c_out)].
            # Prefetched for all 4 pairs during the gram phase; emitted after
            # the memsets so the gpsimd queue stays clear.
            pt8 = {}
            for t, srcpad in enumerate((a8d, b8d)):
                for P in range(4):
                    pt = patp.tile(
                        [128, 2, KK], f8, tag="pat", name=f"pt{t}_{P}"
                    )
                    for m in range(2):
                        c = 2 * P + m
                        nhj = 4 if c < 7 else 3
                        for dh in range(nhj):
                            sap = bass.AP(
                                tensor=srcpad.ap().tensor,
                                offset=(4 * c + dh) * PW * C,
                                ap=[
                                    [C, Wp],
                                    [PW * C, 3],
                                    [C, 3],
                                    [1, C],
                                ],
                            )
                            dma_engs[(2 * P + m + dh) % 2].dma_start(
                                out=pt[32 * dh : 32 * (dh + 1), m, :],
                                in_=sap,
                            )
                    if P == 3:
                        # chunk 7 has only 3 dh rows: zero the tail rows
                        nc.gpsimd.memset(pt[96:128, 1, :], 0.0)
                    pt8[(t, P)] = pt

            with ExitStack() as ph1:
                psD = ph1.enter_context(
                    tc.tile_pool(name="psD", bufs=1, space="PSUM")
                )
                dpsall = psD.tile([1, L], f32, tag="dps", name="dpsall")
                dps = [dpsall[:, i0 : i0 + n] for (i0, n, _, _) in HALves]

                # Gram R = z.T@z per (j-chunk, i-chunk); E symmetric so only
                # i >= 128*jc is computed, rest mirrored by PE transpose.
                def ichunks(jc):
                    off = 128 * jc
                    out = []
                    while off < L:
                        n = min(512, L - off)
                        out.append((off, n))
                        off += n
                    return out

                with tc.tile_pool(name="psR", bufs=6, space="PSUM") as psR:
                    for g0, g1 in ((0, 3), (3, 6), (6, 8)):
                        grp = list(enumerate(JC))[g0:g1]
                        rps = {
                            c: [
                                psR.tile(
                                    [128, n], f32, tag="rps", name=f"rps{c}_{ci}"
                                )
                                for ci, (i0, n) in enumerate(ichunks(c))
                            ]
                            for c, _ in grp
                        }
                        # k-pair-major so early matmuls only need early z8
                        for k in range(9):
                            for c, (j0, dm) in grp:
                                for ci, (i0, n) in enumerate(ichunks(c)):
                                    nc.tensor.matmul(
                                        rps[c][ci][:dm, :],
                                        z8[k][:, :, j0 : j0 + dm],
                                        z8[k][:, :, i0 : i0 + n],
                                        start=(k == 0),
                                        stop=(k == 8),
                                        perf_mode=DR,
                                    )
                        for c, (j0, dm) in grp:
                            t3 = tpad[c].rearrange("j (h w) -> j h w", h=PH, w=PW)
                            for ci, (i0, n) in enumerate(ichunks(c)):
                                h0, nh = i0 // Wp, n // Wp
                                itv = t3[:dm, 1 + h0 : 1 + h0 + nh, 1 : 1 + Wp]
                                nc.vector.tensor_mul(
                                    itv,
                                    rps[c][ci][:dm, :],
                                    sb_inv_b[:dm, i0 : i0 + n],
                                )
                                nc.scalar.activation(
                                    itv,
                                    itv,
                                    mybir.ActivationFunctionType.Exp,
                                    scale=sb_inv_p[:dm, c : c + 1],
                                )

                # mirror lower-triangle blocks, then the softmax denominators
                with tc.tile_pool(name="psT", bufs=2, space="PSUM") as psT, \
                        tc.tile_pool(name="tbp", bufs=3) as tbp:
                    for c, (j0, dm) in enumerate(JC):
                        t3j = tpad[c].rearrange("j (h w) -> j h w", h=PH, w=PW)
                        nhj = dm // Wp
                        for ic in range(c):
                            t3s = tpad[ic].rearrange(
                                "j (h w) -> j h w", h=PH, w=PW
                            )
                            srcv = t3s[:128, 1 + 4 * c : 1 + 4 * c + nhj, 1 : 1 + Wp]
                            tbn = tbp.tile(
                                [128, 128], f32r, tag="tbn", name=f"tbn{c}_{ic}"
                            )
                            nc.vector.tensor_copy(tbn[:, :dm], srcv)
                            pst = psT.tile(
                                [128, 128], f32r, tag="pst", name=f"pst{c}_{ic}"
                            )
                            nc.tensor.transpose(pst[:dm, :128], tbn[:, :dm], idn[:, :])
                            nc.vector.tensor_copy(
                                t3j[:dm, 1 + 4 * ic : 1 + 4 * ic + 4, 1 : 1 + Wp],
                                pst[:dm, :128],
                            )
                        for hi, (i0, n, h0, nh) in enumerate(HALves):
                            nc.tensor.matmul(
                                dps[hi],
                                ones_k[:dm, :],
                                t3j[:dm, 1 + h0 : 1 + h0 + nh, 1 : 1 + Wp],
                                start=(c == 0),
                                stop=(c == 7),
                            )

                # 1/denom into SBUF; psD dies with ph1 before psB opens
                rtmp2 = const.tile([1, L], f32, tag="rtmp2")
                nc.vector.reciprocal_approx_fast(out=rtmp2[:, :], in_=dpsall[:, :])
                nc.vector.tensor_copy(recip_sb[:, :], rtmp2[:, :])

            # Broadcast (x DSC) across partitions via K=1 matmul; the recon
            # normalize reads it directly from PSUM (bank reserved through
            # the recon phase: 2 banks here + 6 psY banks = 8).
            psB = ctx.enter_context(
                tc.tile_pool(name="psB", bufs=1, space="PSUM")
            )
            bpsall = psB.tile([128, L], f32, tag="bps", name="bpsall")
            for hi, (i0, n, _, _) in enumerate(HALves):
                nc.tensor.matmul(
                    bpsall[:, i0 : i0 + n],
                    sc_m[:, :],
                    recip_sb[:, i0 : i0 + n],
                    start=True,
                    stop=True,
                )

            # Reconstruction of the deviation term, DoubleRow over pairs.
            # Outputs accumulate in grid-shaped PSUM tiles (rows of 34 incl
            # 2 junk cols) so each (p,q)-shifted rhs is a single contiguous
            # [j, 2, nh*34] stream (the DoubleRow-compatible 3D form). Two
            # sequential sweeps (tensor a then b), 6 PSUM banks each; sweep
            # a's output copy/DMA overlaps sweep b's matmuls.
            # Per chunk: normalize E by DSC*recip_i (DVE), then cast to fp8
            # with the uniform offset folded into the activation bias.
            SECS = [(0, 11), (11, 10), (21, 10)]  # (h0, nh) over l' rows
            with ExitStack() as ph2:
                psY = ph2.enter_context(
                    tc.tile_pool(name="psY", bufs=6, space="PSUM")
                )
                for t, dram in enumerate((ya_t, yb_t)):
                    yps = [
                        [
                            psY.tile(
                                [128, nh * PW],
                                f32,
                                tag="yps",
                                name=f"yps{t}_{cb}_{si}",
                            )
                            for si, (h0, nh) in enumerate(SECS)
                        ]
                        for cb in range(2)
                    ]
                    for P in range(4):
                        if t == 0:
                            s4 = s8[P][:, :, :NPAD].rearrange(
                                "j m (h w) -> j m h w", h=PH, w=PW
                            )
                            for m in range(2):
                                c = 2 * P + m
                                j0, dm = JC[c]
                                t3 = tpad[c].rearrange(
                                    "j (h w) -> j h w", h=PH, w=PW
                                )
                                for hi, (i0, n, h0, nh) in enumerate(HALves):
                                    itv = t3[
                                        :dm, 1 + h0 : 1 + h0 + nh, 1 : 1 + Wp
                                    ]
                                    nc.vector.tensor_mul(
                                        itv, itv, bpsall[:dm, i0 : i0 + n]
                                    )
                                    nc.scalar.activation(
                                        s4[:dm, m, 1 + h0 : 1 + h0 + nh, 1 : 1 + Wp],
                                        itv,
                                        mybir.ActivationFunctionType.Identity,
                                        bias=bias_u[:dm, :],
                                    )
                        for p in range(3):
                            for q in range(3):
                                for cb in range(2):
                                    lhs = pt8[(t, P)][
                                        :,
                                        :,
                                        (3 * p + q) * C
                                        + 128 * cb : (3 * p + q) * C
                                        + 128 * (cb + 1),
                                    ]
                                    for si, (h0, nh) in enumerate(SECS):
                                        g0 = (h0 - p + 2) * PW + (2 - q)
                                        nc.tensor.matmul(
                                            yps[cb][si][:, :],
                                            lhs,
                                            s8[P][:, :, g0 : g0 + nh * PW],
                                            start=(P == 0 and p == 0 and q == 0),
                                            stop=(P == 3 and p == 2 and q == 2),
                                            perf_mode=DR,
                                        )
                    for cb in range(2):
                        ysb = outp.tile(
                            [128, L], bf16, tag="ysb", name=f"ysb{t}_{cb}"
                        )
                        for si, (h0, nh) in enumerate(SECS):
                            ypv = yps[cb][si].rearrange(
                                "c (h w) -> c h w", h=nh, w=PW
                            )[:, :, 0:Wp]
                            ysv = ysb[:, h0 * Wp : (h0 + nh) * Wp].rearrange(
                                "c (h w) -> c h w", h=nh, w=Wp
                            )
                            nc.vector.tensor_copy(ysv, ypv)
                        [nc.sync, nc.scalar, nc.sync, nc.scalar][
                            2 * t + cb
                        ].dma_start(
                            out=dram[128 * cb : 128 * (cb + 1), :], in_=ysb[:]
                        )

    nc.compile()
    return nc


def _get_program():
    if "nc" not in _CACHE:
        _CACHE["nc"] = _build_program()
    return _CACHE["nc"]


def _prep_core(A, B):
    """A, B: [31,32,256] float32 -> (input map, host uniform term [L, C])."""
    ap = np.zeros((PH, PW, C), np.float32)
    ap[1 : 1 + Hp, 1 : 1 + Wp] = A
    bp = np.zeros((PH, PW, C), np.float32)
    bp[1 : 1 + Hp, 1 : 1 + Wp] = B

    # patches [3,3,C,L] without materializing: strided windows
    def win(pad, p, q):
        return pad[p : p + Hp, q : q + Wp]  # [Hp, Wp, C]

    ss_a = np.zeros((Hp, Wp))
    ss_b = np.zeros((Hp, Wp))
    z8 = np.empty((1152, 2 * L), dtype=E4)
    zrow = np.empty((C, L), np.float32)
    for p in range(3):
        for q in range(3):
            wa = win(ap, p, q).astype(np.float64)
            wb = win(bp, p, q).astype(np.float64)
            ss_a += (wa * wa).sum(-1)
            ss_b += (wb * wb).sum(-1)
            np.multiply(
                win(ap, p, q).reshape(L, C).T,
                win(bp, p, q).reshape(L, C).T,
                out=zrow,
            )
            kk = 2 * (3 * p + q)  # two 128-row slices per (p,q)
            for half in range(2):
                rows = zrow[128 * half : 128 * (half + 1)]
                pair, mm = divmod(kk + half, 2)
                z8[128 * pair : 128 * (pair + 1), mm * L : (mm + 1) * L] = (
                    ZSC * rows
                ).astype(E4)
    inv = (
        1.0
        / np.maximum(np.sqrt(ss_a), 1e-4)
        / np.maximum(np.sqrt(ss_b), 1e-4)
    ).reshape(-1)

    # host uniform term: y_mean[l', c] = u * sum_pq mask * window-sum
    u = 1.0 / L
    Ug = np.zeros((PH, PW))
    Ug[1 : 1 + Hp, 1 : 1 + Wp] = 1.0
    ymean_a = np.zeros((L, C))
    ymean_b = np.zeros((L, C))
    for p in range(3):
        for q in range(3):
            w = Ug[2 - p : 2 - p + Hp, 2 - q : 2 - q + Wp].reshape(L, 1)
            ymean_a += u * w @ win(ap, p, q).astype(np.float64).sum((0, 1))[None, :]
            ymean_b += u * w @ win(bp, p, q).astype(np.float64).sum((0, 1))[None, :]

    inp = {
        "z8": z8,
        "a8": (PSC * ap).astype(E4),
        "b8": (PSC * bp).astype(E4),
        "inv_p": np.ascontiguousarray(
            np.pad(10.0 * inv, (0, 1024 - L)).reshape(8, 128).T.astype(np.float32)
        ),
        "inv_f": (inv / (ZSC * ZSC)).reshape(1, L).astype(np.float32),
    }
    return inp, ymean_a, ymean_b


def _assemble(res, ymean_a, ymean_b):
    """Device bf16 deviation outputs [C, L] -> full [Hp, Wp, C] pair."""
    sc = 1.0 / (DSC * PSC)
    ya = ymean_a + sc * res["ya_t"].astype(np.float64).T
    yb = ymean_b + sc * res["yb_t"].astype(np.float64).T
    return (
        ya.reshape(Hp, Wp, C).astype(np.float32),
        yb.reshape(Hp, Wp, C).astype(np.float32),
    )


def kernel(x, mask):
    x = np.asarray(x, dtype=np.float32)
    in_maps = []
    hosts = []
    for b in range(B_IMG):
        xb = x[b]
        im, ha, hb = _prep_core(xb[:-1], xb[1:])
        in_maps.append(im)
        hosts.append((ha, hb))
        xt = np.ascontiguousarray(xb.transpose(1, 0, 2))
        im, ha, hb = _prep_core(xt[1:], xt[:-1])
        in_maps.append(im)
        hosts.append((ha, hb))

    from concourse.bass_utils import run_bass_kernel_spmd

    nc = _get_program()
    res = run_bass_kernel_spmd(nc, in_maps, list(range(8))).results

    out = np.empty((B_IMG, H_IMG, W_IMG, C), np.float32)
    for b in range(B_IMG):
        yl, yr = _assemble(res[2 * b], *hosts[2 * b])
        ylr = np.concatenate(
            [yr[:1], (yr[1:] + yl[:-1]) * 0.5, yl[-1:]], axis=0
        )
        yt, yb = _assemble(res[2 * b + 1], *hosts[2 * b + 1])
        yt = yt.transpose(1, 0, 2)
        yb = yb.transpose(1, 0, 2)
        ytb = np.concatenate(
            [yt[:, :1], (yt[:, 1:] + yb[:, :-1]) * 0.5, yb[:, -1:]], axis=1
        )
        out[b] = (ylr + ytb) * 0.5
    return out


# revision 16
# speedup vs baseline: 1.7614x; 1.1394x over previous
"""EnvironmentConsistentAttention on 8 trn2 cores — fp8 DoubleRow version.

Sharding: 4 images x 2 directions (vertical/horizontal neighbor pairs) = 8
independent units, one per core. The horizontal direction of image x equals
the vertical direction of x spatially transposed, so a single SPMD program
handles both: given shifted maps A, B [31,32,256] it returns the fp8
*deviation* reconstruction; the exact uniform-attention part is added on the
host.

Math per core (Hp=31, Wp=32, C=256, L=992, k=3):
  pa[(p,q,c), l] = A_pad[h+p, w+q, c];  z = pa*pb  [2304, L]
  R = z.T @ z;  att = 10*inv_i*inv_j*R;  S = softmax(att, axis=j)
  y = conv_transpose(S, pa)  (and pb)

Key numeric fact for this problem: S is extremely close to uniform (u=1/L),
and y is dominated by the rank-1 uniform term. So split S = u + D and
compute only the deviation term on the accelerator with fp8e4m3 DoubleRow
matmuls (K=256 per pass at 0.5 cycles/row = 4x f32r throughput):
  - gram: z quantized to fp8 on host (scale 4), 9 partition-pair matmul
    groups; exp/softmax-denominator stay f32 (as in the f32r kernel:
    symmetric upper-triangle + PE-transpose mirror; ones-matmul colsums).
  - recon: D = (recip_i*E_ij - u) scaled by 1024, cast to fp8 on the ACT
    engine (Identity activation with bias=-1024/L), patches fp8 from host
    (scale 16). DoubleRow over 4 j-chunk-pairs.
  - uniform term: y_mean[l',c] = u * sum_pq mask(l',p,q) * window_sum(pad)
    computed exactly on host and added back; device output is bf16 (it only
    carries the small deviation term).
Measured model error of this scheme vs the jax reference: l2 ~4.4e-5.
"""

import numpy as np
import ml_dtypes

Hp, Wp, C = 31, 32, 256
L = Hp * Wp            # 992
PH, PW = Hp + 2, Wp + 2  # 33, 34 padded grid
NPAD = PH * PW         # 1122
KK = 9 * C             # 2304
JC = [(128 * c, 128 if c < 7 else 96) for c in range(8)]   # j/l chunks
HALves = [(0, 512, 0, 16), (512, 480, 16, 15)]  # (i0, n, h0, nh) over i/l'
B_IMG, H_IMG, W_IMG = 4, 32, 32

ZSC = 4.0       # host z fp8 scale (per factor; gram R is scaled by ZSC^2)
PSC = 16.0      # host patch fp8 scale
DSC = 1024.0    # device D fp8 scale
E4 = ml_dtypes.float8_e4m3

_CACHE = {}


def _build_program():
    import concourse.bass as bass
    import concourse.tile as tile
    from concourse import bacc, mybir

    f32 = mybir.dt.float32
    f32r = mybir.dt.float32r
    f8 = mybir.dt.float8e4
    bf16 = mybir.dt.bfloat16
    DR = mybir.MatmulPerfMode.DoubleRow

    nc = bacc.Bacc("TRN2", target_bir_lowering=False, debug=False)

    z8d = nc.dram_tensor("z8", [1152, 2 * L], f8, kind="ExternalInput")
    a8d = nc.dram_tensor("a8", [PH, PW, C], f8, kind="ExternalInput")
    b8d = nc.dram_tensor("b8", [PH, PW, C], f8, kind="ExternalInput")
    inv_p = nc.dram_tensor("inv_p", [128, 8], f32, kind="ExternalInput")
    inv_f = nc.dram_tensor("inv_f", [1, L], f32, kind="ExternalInput")
    ya_t = nc.dram_tensor("ya_t", [C, L], bf16, kind="ExternalOutput")
    yb_t = nc.dram_tensor("yb_t", [C, L], bf16, kind="ExternalOutput")

    with tile.TileContext(nc) as tc:
        from contextlib import ExitStack

        with ExitStack() as ctx:
            const = ctx.enter_context(tc.tile_pool(name="const", bufs=1))
            outp = ctx.enter_context(tc.tile_pool(name="outp", bufs=4))
            tpadp = ctx.enter_context(tc.tile_pool(name="tpad", bufs=8))
            z8p = ctx.enter_context(tc.tile_pool(name="z8p", bufs=9))
            patp = ctx.enter_context(tc.tile_pool(name="pat", bufs=8))
            s8p = ctx.enter_context(tc.tile_pool(name="s8p", bufs=4))

            # gpsimd dma_start goes through a slow DIRECT2D path and clogs
            # the gpsimd sequencer (whose memsets gate the exp/cast chain):
            # only sync and scalar issue DMAs.
            dma_engs = [nc.sync, nc.scalar]

            # z8 pair tiles first: the gram chases these
            z8 = []
            for k in range(9):
                zt = z8p.tile([128, 2, L], f8, tag="z8", name=f"z8_{k}")
                dma_engs[k % 2].dma_start(
                    out=zt[:], in_=z8d[128 * k : 128 * (k + 1), :]
                )
                z8.append(zt)

            # Constants
            sb_inv_p = const.tile([128, 8], f32, tag="invp")
            sb_inv_b = const.tile([128, L], f32, tag="invb")
            nc.sync.dma_start(out=sb_inv_p[:], in_=inv_p[:, :])
            nc.sync.dma_start(
                out=sb_inv_b[:], in_=inv_f.ap().to_broadcast([128, L])
            )
            ones_f = const.tile([128, 128], f32, tag="onesf")
            nc.vector.memset(ones_f[:], 1.0)
            ones_k = const.tile([128, 1], f32r, tag="onesk")
            nc.scalar.copy(ones_k[:], ones_f[:, 0:1])
            sc_f = const.tile([1, 128], f32, tag="scf")
            nc.vector.memset(sc_f[:], DSC)
            sc_m = const.tile([1, 128], f32r, tag="scm")
            nc.scalar.copy(sc_m[:], sc_f[:, :])
            bias_u = const.tile([128, 1], f32, tag="biasu")
            nc.vector.memset(bias_u[:], -DSC / L)
            from concourse.masks import make_identity

            idn_f = const.tile([128, 128], f32, tag="idnf")
            idn = const.tile([128, 128], f32r, tag="idn")
            make_identity(nc, idn_f[:])
            nc.scalar.copy(idn[:], idn_f[:])
            recip_sb = const.tile([1, L], f32r, tag="recip")

            # S.T deviation tiles (fp8, padded grid + 2 overrun cols) per
            # j-chunk pair. Recon reads them as flat [j, m, nh*34] streams.
            s8 = [
                s8p.tile([128, 2, NPAD + 2], f8, tag="s8", name=f"s8_{P}")
                for P in range(4)
            ]
            for P in range(4):
                t4 = s8[P][:, :, :NPAD].rearrange(
                    "j m (h w) -> j m h w", h=PH, w=PW
                )
                for m in range(2):
                    nc.gpsimd.memset(t4[:, m, 0:1, :], 0.0)
                    nc.gpsimd.memset(t4[:, m, PH - 1 : PH, :], 0.0)
                    nc.gpsimd.memset(t4[:, m, :, 0:1], 0.0)
                    nc.gpsimd.memset(t4[:, m, :, PW - 1 : PW], 0.0)
                nc.gpsimd.memset(s8[P][:, :, NPAD : NPAD + 2], 0.0)
            # chunk 7 has dm=96: zero its tail partitions once
            nc.gpsimd.memset(s8[3][96:128, 1, :], 0.0)

            # E tiles in padded-grid layout (f32r), zeroed borders
            tpad = [
                tpadp.tile([128, NPAD], f32r, tag="tpad", name=f"tpad{c}")
                for c in range(8)
            ]
            for c in range(8):
                tf = tpad[c].bitcast(f32).rearrange(
                    "j (h w) -> j h w", h=PH, w=PW
                )
                nc.gpsimd.memset(tf[:, 0:1, :], 0.0)
                nc.gpsimd.memset(tf[:, PH - 1 : PH, :], 0.0)
                nc.gpsimd.memset(tf[:, :, 0:1], 0.0)
                nc.gpsimd.memset(tf[:, :, PW - 1 : PW], 0.0)

            # Patch-gather tiles (fp8): [j-part, pair-member, (p,q,c_out)].
            # Prefetched during the gram phase. Each gather dma_start costs
            # ~600ns on its issuing sequencer, so all 64 go on gpsimd (idle
            # after the memsets) to keep sync/scalar queues clear for the
            # exp/recip/cast chain.
            pt8 = {}
            for t, srcpad in enumerate((a8d, b8d)):
                for P in range(4):
                    pt = patp.tile(
                        [128, 2, KK], f8, tag="pat", name=f"pt{t}_{P}"
                    )
                    for m in range(2):
                        c = 2 * P + m
                        nhj = 4 if c < 7 else 3
                        for dh in range(nhj):
                            sap = bass.AP(
                                tensor=srcpad.ap().tensor,
                                offset=(4 * c + dh) * PW * C,
                                ap=[
                                    [C, Wp],
                                    [PW * C, 3],
                                    [C, 3],
                                    [1, C],
                                ],
                            )
                            nc.gpsimd.dma_start(
                                out=pt[32 * dh : 32 * (dh + 1), m, :],
                                in_=sap,
                            )
                    if P == 3:
                        # chunk 7 has only 3 dh rows: zero the tail rows
                        nc.gpsimd.memset(pt[96:128, 1, :], 0.0)
                    pt8[(t, P)] = pt

            with ExitStack() as ph1:
                psD = ph1.enter_context(
                    tc.tile_pool(name="psD", bufs=1, space="PSUM")
                )
                dpsall = psD.tile([1, L], f32, tag="dps", name="dpsall")
                dps = [dpsall[:, i0 : i0 + n] for (i0, n, _, _) in HALves]

                # Gram R = z.T@z per (j-chunk, i-chunk); E symmetric so only
                # i >= 128*jc is computed, rest mirrored by PE transpose.
                def ichunks(jc):
                    off = 128 * jc
                    out = []
                    while off < L:
                        n = min(512, L - off)
                        out.append((off, n))
                        off += n
                    return out

                with tc.tile_pool(name="psR", bufs=6, space="PSUM") as psR:
                    for g0, g1 in ((0, 3), (3, 6), (6, 8)):
                        grp = list(enumerate(JC))[g0:g1]
                        rps = {
                            c: [
                                psR.tile(
                                    [128, n], f32, tag="rps", name=f"rps{c}_{ci}"
                                )
                                for ci, (i0, n) in enumerate(ichunks(c))
                            ]
                            for c, _ in grp
                        }
                        # k-pair-major so early matmuls only need early z8
                        for k in range(9):
                            for c, (j0, dm) in grp:
                                for ci, (i0, n) in enumerate(ichunks(c)):
                                    nc.tensor.matmul(
                                        rps[c][ci][:dm, :],
                                        z8[k][:, :, j0 : j0 + dm],
                                        z8[k][:, :, i0 : i0 + n],
                                        start=(k == 0),
                                        stop=(k == 8),
                                        perf_mode=DR,
                                    )
                        for c, (j0, dm) in grp:
                            t3 = tpad[c].rearrange("j (h w) -> j h w", h=PH, w=PW)
                            for ci, (i0, n) in enumerate(ichunks(c)):
                                h0, nh = i0 // Wp, n // Wp
                                itv = t3[:dm, 1 + h0 : 1 + h0 + nh, 1 : 1 + Wp]
                                nc.vector.tensor_mul(
                                    itv,
                                    rps[c][ci][:dm, :],
                                    sb_inv_b[:dm, i0 : i0 + n],
                                )
                                nc.scalar.activation(
                                    itv,
                                    itv,
                                    mybir.ActivationFunctionType.Exp,
                                    scale=sb_inv_p[:dm, c : c + 1],
                                )

                # mirror lower-triangle blocks, then the softmax denominators
                with tc.tile_pool(name="psT", bufs=4, space="PSUM") as psT, \
                        tc.tile_pool(name="tbp", bufs=6) as tbp:
                    for c, (j0, dm) in enumerate(JC):
                        t3j = tpad[c].rearrange("j (h w) -> j h w", h=PH, w=PW)
                        nhj = dm // Wp
                        for ic in range(c):
                            t3s = tpad[ic].rearrange(
                                "j (h w) -> j h w", h=PH, w=PW
                            )
                            srcv = t3s[:128, 1 + 4 * c : 1 + 4 * c + nhj, 1 : 1 + Wp]
                            tbn = tbp.tile(
                                [128, 128], f32r, tag="tbn", name=f"tbn{c}_{ic}"
                            )
                            nc.vector.tensor_copy(tbn[:, :dm], srcv)
                            pst = psT.tile(
                                [128, 128], f32r, tag="pst", name=f"pst{c}_{ic}"
                            )
                            nc.tensor.transpose(pst[:dm, :128], tbn[:, :dm], idn[:, :])
                            nc.vector.tensor_copy(
                                t3j[:dm, 1 + 4 * ic : 1 + 4 * ic + 4, 1 : 1 + Wp],
                                pst[:dm, :128],
                            )
                        for hi, (i0, n, h0, nh) in enumerate(HALves):
                            nc.tensor.matmul(
                                dps[hi],
                                ones_k[:dm, :],
                                t3j[:dm, 1 + h0 : 1 + h0 + nh, 1 : 1 + Wp],
                                start=(c == 0),
                                stop=(c == 7),
                            )

                # 1/denom into SBUF; psD dies with ph1 before psB opens
                rtmp2 = const.tile([1, L], f32, tag="rtmp2")
                nc.vector.reciprocal_approx_fast(out=rtmp2[:, :], in_=dpsall[:, :])
                nc.vector.tensor_copy(recip_sb[:, :], rtmp2[:, :])

            # Broadcast (x DSC) across partitions via K=1 matmul; the recon
            # normalize reads it directly from PSUM (bank reserved through
            # the recon phase: 2 banks here + 6 psY banks = 8).
            psB = ctx.enter_context(
                tc.tile_pool(name="psB", bufs=1, space="PSUM")
            )
            bpsall = psB.tile([128, L], f32, tag="bps", name="bpsall")
            for hi, (i0, n, _, _) in enumerate(HALves):
                nc.tensor.matmul(
                    bpsall[:, i0 : i0 + n],
                    sc_m[:, :],
                    recip_sb[:, i0 : i0 + n],
                    start=True,
                    stop=True,
                )

            # Reconstruction of the deviation term, DoubleRow over pairs.
            # Outputs accumulate in grid-shaped PSUM tiles (rows of 34 incl
            # 2 junk cols) so each (p,q)-shifted rhs is a single contiguous
            # [j, 2, nh*34] stream (the DoubleRow-compatible 3D form). Two
            # sequential sweeps (tensor a then b), 6 PSUM banks each; sweep
            # a's output copy/DMA overlaps sweep b's matmuls.
            # Per chunk: normalize E by DSC*recip_i (DVE), then cast to fp8
            # with the uniform offset folded into the activation bias.
            SECS = [(0, 11), (11, 10), (21, 10)]  # (h0, nh) over l' rows
            with ExitStack() as ph2:
                psY = ph2.enter_context(
                    tc.tile_pool(name="psY", bufs=6, space="PSUM")
                )
                for t, dram in enumerate((ya_t, yb_t)):
                    yps = [
                        [
                            psY.tile(
                                [128, nh * PW],
                                f32,
                                tag="yps",
                                name=f"yps{t}_{cb}_{si}",
                            )
                            for si, (h0, nh) in enumerate(SECS)
                        ]
                        for cb in range(2)
                    ]
                    for P in range(4):
                        if t == 0:
                            s4 = s8[P][:, :, :NPAD].rearrange(
                                "j m (h w) -> j m h w", h=PH, w=PW
                            )
                            for m in range(2):
                                c = 2 * P + m
                                j0, dm = JC[c]
                                t3 = tpad[c].rearrange(
                                    "j (h w) -> j h w", h=PH, w=PW
                                )
                                for hi, (i0, n, h0, nh) in enumerate(HALves):
                                    itv = t3[
                                        :dm, 1 + h0 : 1 + h0 + nh, 1 : 1 + Wp
                                    ]
                                    nc.vector.tensor_mul(
                                        itv, itv, bpsall[:dm, i0 : i0 + n]
                                    )
                                    nc.scalar.activation(
                                        s4[:dm, m, 1 + h0 : 1 + h0 + nh, 1 : 1 + Wp],
                                        itv,
                                        mybir.ActivationFunctionType.Identity,
                                        bias=bias_u[:dm, :],
                                    )
                        for p in range(3):
                            for q in range(3):
                                for cb in range(2):
                                    lhs = pt8[(t, P)][
                                        :,
                                        :,
                                        (3 * p + q) * C
                                        + 128 * cb : (3 * p + q) * C
                                        + 128 * (cb + 1),
                                    ]
                                    for si, (h0, nh) in enumerate(SECS):
                                        g0 = (h0 - p + 2) * PW + (2 - q)
                                        nc.tensor.matmul(
                                            yps[cb][si][:, :],
                                            lhs,
                                            s8[P][:, :, g0 : g0 + nh * PW],
                                            start=(P == 0 and p == 0 and q == 0),
                                            stop=(P == 3 and p == 2 and q == 2),
                                            perf_mode=DR,
                                        )
                    for cb in range(2):
                        ysb = outp.tile(
                            [128, L], bf16, tag="ysb", name=f"ysb{t}_{cb}"
                        )
                        for si, (h0, nh) in enumerate(SECS):
                            ypv = yps[cb][si].rearrange(
                                "c (h w) -> c h w", h=nh, w=PW
                            )[:, :, 0:Wp]
                            ysv = ysb[:, h0 * Wp : (h0 + nh) * Wp].rearrange(
                                "c (h w) -> c h w", h=nh, w=Wp
                            )
                            nc.vector.tensor_copy(ysv, ypv)
                        [nc.sync, nc.scalar, nc.sync, nc.scalar][
                            2 * t + cb
                        ].dma_start(
                            out=dram[128 * cb : 128 * (cb + 1), :], in_=ysb[:]
                        )

    nc.compile()
    return nc


def _get_program():
    if "nc" not in _CACHE:
        _CACHE["nc"] = _build_program()
    return _CACHE["nc"]


def _prep_core(A, B):
    """A, B: [31,32,256] float32 -> (input map, host uniform term [L, C])."""
    ap = np.zeros((PH, PW, C), np.float32)
    ap[1 : 1 + Hp, 1 : 1 + Wp] = A
    bp = np.zeros((PH, PW, C), np.float32)
    bp[1 : 1 + Hp, 1 : 1 + Wp] = B

    # patches [3,3,C,L] without materializing: strided windows
    def win(pad, p, q):
        return pad[p : p + Hp, q : q + Wp]  # [Hp, Wp, C]

    ss_a = np.zeros((Hp, Wp))
    ss_b = np.zeros((Hp, Wp))
    z8 = np.empty((1152, 2 * L), dtype=E4)
    zrow = np.empty((C, L), np.float32)
    for p in range(3):
        for q in range(3):
            wa = win(ap, p, q).astype(np.float64)
            wb = win(bp, p, q).astype(np.float64)
            ss_a += (wa * wa).sum(-1)
            ss_b += (wb * wb).sum(-1)
            np.multiply(
                win(ap, p, q).reshape(L, C).T,
                win(bp, p, q).reshape(L, C).T,
                out=zrow,
            )
            kk = 2 * (3 * p + q)  # two 128-row slices per (p,q)
            for half in range(2):
                rows = zrow[128 * half : 128 * (half + 1)]
                pair, mm = divmod(kk + half, 2)
                z8[128 * pair : 128 * (pair + 1), mm * L : (mm + 1) * L] = (
                    ZSC * rows
                ).astype(E4)
    inv = (
        1.0
        / np.maximum(np.sqrt(ss_a), 1e-4)
        / np.maximum(np.sqrt(ss_b), 1e-4)
    ).reshape(-1)

    # host uniform term: y_mean[l', c] = u * sum_pq mask * window-sum
    u = 1.0 / L
    Ug = np.zeros((PH, PW))
    Ug[1 : 1 + Hp, 1 : 1 + Wp] = 1.0
    ymean_a = np.zeros((L, C))
    ymean_b = np.zeros((L, C))
    for p in range(3):
        for q in range(3):
            w = Ug[2 - p : 2 - p + Hp, 2 - q : 2 - q + Wp].reshape(L, 1)
            ymean_a += u * w @ win(ap, p, q).astype(np.float64).sum((0, 1))[None, :]
            ymean_b += u * w @ win(bp, p, q).astype(np.float64).sum((0, 1))[None, :]

    inp = {
        "z8": z8,
        "a8": (PSC * ap).astype(E4),
        "b8": (PSC * bp).astype(E4),
        "inv_p": np.ascontiguousarray(
            np.pad(10.0 * inv, (0, 1024 - L)).reshape(8, 128).T.astype(np.float32)
        ),
        "inv_f": (inv / (ZSC * ZSC)).reshape(1, L).astype(np.float32),
    }
    return inp, ymean_a, ymean_b


def _assemble(res, ymean_a, ymean_b):
    """Device bf16 deviation outputs [C, L] -> full [Hp, Wp, C] pair."""
    sc = 1.0 / (DSC * PSC)
    ya = ymean_a + sc * res["ya_t"].astype(np.float64).T
    yb = ymean_b + sc * res["yb_t"].astype(np.float64).T
    return (
        ya.reshape(Hp, Wp, C).astype(np.float32),
        yb.reshape(Hp, Wp, C).astype(np.float32),
    )


def kernel(x, mask):
    x = np.asarray(x, dtype=np.float32)
    in_maps = []
    hosts = []
    for b in range(B_IMG):
        xb = x[b]
        im, ha, hb = _prep_core(xb[:-1], xb[1:])
        in_maps.append(im)
        hosts.append((ha, hb))
        xt = np.ascontiguousarray(xb.transpose(1, 0, 2))
        im, ha, hb = _prep_core(xt[1:], xt[:-1])
        in_maps.append(im)
        hosts.append((ha, hb))

    from concourse.bass_utils import run_bass_kernel_spmd

    nc = _get_program()
    res = run_bass_kernel_spmd(nc, in_maps, list(range(8))).results

    out = np.empty((B_IMG, H_IMG, W_IMG, C), np.float32)
    for b in range(B_IMG):
        yl, yr = _assemble(res[2 * b], *hosts[2 * b])
        ylr = np.concatenate(
            [yr[:1], (yr[1:] + yl[:-1]) * 0.5, yl[-1:]], axis=0
        )
        yt, yb = _assemble(res[2 * b + 1], *hosts[2 * b + 1])
        yt = yt.transpose(1, 0, 2)
        yb = yb.transpose(1, 0, 2)
        ytb = np.concatenate(
            [yt[:, :1], (yt[:, 1:] + yb[:, :-1]) * 0.5, yb[:, -1:]], axis=1
        )
        out[b] = (ylr + ytb) * 0.5
    return out


# revision 17
# speedup vs baseline: 1.7677x; 1.0036x over previous
"""EnvironmentConsistentAttention on 8 trn2 cores — fp8 DoubleRow version.

Sharding: 4 images x 2 directions (vertical/horizontal neighbor pairs) = 8
independent units, one per core. The horizontal direction of image x equals
the vertical direction of x spatially transposed, so a single SPMD program
handles both: given shifted maps A, B [31,32,256] it returns the fp8
*deviation* reconstruction; the exact uniform-attention part is added on the
host.

Math per core (Hp=31, Wp=32, C=256, L=992, k=3):
  pa[(p,q,c), l] = A_pad[h+p, w+q, c];  z = pa*pb  [2304, L]
  R = z.T @ z;  att = 10*inv_i*inv_j*R;  S = softmax(att, axis=j)
  y = conv_transpose(S, pa)  (and pb)

Key numeric fact for this problem: S is extremely close to uniform (u=1/L),
and y is dominated by the rank-1 uniform term. So split S = u + D and
compute only the deviation term on the accelerator with fp8e4m3 DoubleRow
matmuls (K=256 per pass at 0.5 cycles/row = 4x f32r throughput):
  - gram: z quantized to fp8 on host (scale 4), 9 partition-pair matmul
    groups; exp/softmax-denominator stay f32 (as in the f32r kernel:
    symmetric upper-triangle + PE-transpose mirror; ones-matmul colsums).
  - recon: D = (recip_i*E_ij - u) scaled by 1024, cast to fp8 on the ACT
    engine (Identity activation with bias=-1024/L), patches fp8 from host
    (scale 16). DoubleRow over 4 j-chunk-pairs.
  - uniform term: y_mean[l',c] = u * sum_pq mask(l',p,q) * window_sum(pad)
    computed exactly on host and added back; device output is bf16 (it only
    carries the small deviation term).
Measured model error of this scheme vs the jax reference: l2 ~4.4e-5.
"""

import numpy as np
import ml_dtypes

Hp, Wp, C = 31, 32, 256
L = Hp * Wp            # 992
PH, PW = Hp + 2, Wp + 2  # 33, 34 padded grid
NPAD = PH * PW         # 1122
KK = 9 * C             # 2304
JC = [(128 * c, 128 if c < 7 else 96) for c in range(8)]   # j/l chunks
HALves = [(0, 512, 0, 16), (512, 480, 16, 15)]  # (i0, n, h0, nh) over i/l'
B_IMG, H_IMG, W_IMG = 4, 32, 32

ZSC = 4.0       # host z fp8 scale (per factor; gram R is scaled by ZSC^2)
PSC = 16.0      # host patch fp8 scale
DSC = 1024.0    # device D fp8 scale
E4 = ml_dtypes.float8_e4m3

_CACHE = {}


def _build_program():
    import concourse.bass as bass
    import concourse.tile as tile
    from concourse import bacc, mybir

    f32 = mybir.dt.float32
    f32r = mybir.dt.float32r
    f8 = mybir.dt.float8e4
    bf16 = mybir.dt.bfloat16
    DR = mybir.MatmulPerfMode.DoubleRow

    nc = bacc.Bacc("TRN2", target_bir_lowering=False, debug=False)

    z8d = nc.dram_tensor("z8", [1152, 2 * L], f8, kind="ExternalInput")
    a8d = nc.dram_tensor("a8", [PH, PW, C], f8, kind="ExternalInput")
    b8d = nc.dram_tensor("b8", [PH, PW, C], f8, kind="ExternalInput")
    inv_p = nc.dram_tensor("inv_p", [128, 8], f32, kind="ExternalInput")
    inv_f = nc.dram_tensor("inv_f", [1, L], f32, kind="ExternalInput")
    ya_t = nc.dram_tensor("ya_t", [C, L], bf16, kind="ExternalOutput")
    yb_t = nc.dram_tensor("yb_t", [C, L], bf16, kind="ExternalOutput")

    with tile.TileContext(nc) as tc:
        from contextlib import ExitStack

        with ExitStack() as ctx:
            const = ctx.enter_context(tc.tile_pool(name="const", bufs=1))
            outp = ctx.enter_context(tc.tile_pool(name="outp", bufs=4))
            tpadp = ctx.enter_context(tc.tile_pool(name="tpad", bufs=8))
            z8p = ctx.enter_context(tc.tile_pool(name="z8p", bufs=9))
            patp = ctx.enter_context(tc.tile_pool(name="pat", bufs=8))
            s8p = ctx.enter_context(tc.tile_pool(name="s8p", bufs=4))

            # gpsimd dma_start goes through a slow DIRECT2D path and clogs
            # the gpsimd sequencer (whose memsets gate the exp/cast chain):
            # only sync and scalar issue DMAs.
            dma_engs = [nc.sync, nc.scalar]

            # z8 pair tiles first: the gram chases these
            z8 = []
            for k in range(9):
                zt = z8p.tile([128, 2, L], f8, tag="z8", name=f"z8_{k}")
                dma_engs[k % 2].dma_start(
                    out=zt[:], in_=z8d[128 * k : 128 * (k + 1), :]
                )
                z8.append(zt)

            # Constants
            sb_inv_p = const.tile([128, 8], f32, tag="invp")
            sb_inv_b = const.tile([128, L], f32, tag="invb")
            nc.sync.dma_start(out=sb_inv_p[:], in_=inv_p[:, :])
            nc.sync.dma_start(
                out=sb_inv_b[:], in_=inv_f.ap().to_broadcast([128, L])
            )
            ones_f = const.tile([128, 128], f32, tag="onesf")
            nc.vector.memset(ones_f[:], 1.0)
            ones_k = const.tile([128, 1], f32r, tag="onesk")
            nc.scalar.copy(ones_k[:], ones_f[:, 0:1])
            sc_f = const.tile([1, 128], f32, tag="scf")
            nc.vector.memset(sc_f[:], DSC)
            sc_m = const.tile([1, 128], f32r, tag="scm")
            nc.scalar.copy(sc_m[:], sc_f[:, :])
            bias_u = const.tile([128, 1], f32, tag="biasu")
            nc.vector.memset(bias_u[:], -DSC / L)
            from concourse.masks import make_identity

            idn_f = const.tile([128, 128], f32, tag="idnf")
            idn = const.tile([128, 128], f32r, tag="idn")
            make_identity(nc, idn_f[:])
            nc.scalar.copy(idn[:], idn_f[:])
            recip_sb = const.tile([1, L], f32r, tag="recip")

            # S.T deviation tiles (fp8, padded grid + 2 overrun cols) per
            # j-chunk pair. Recon reads them as flat [j, m, nh*34] streams.
            s8 = [
                s8p.tile([128, 2, NPAD + 2], f8, tag="s8", name=f"s8_{P}")
                for P in range(4)
            ]
            for P in range(4):
                t4 = s8[P][:, :, :NPAD].rearrange(
                    "j m (h w) -> j m h w", h=PH, w=PW
                )
                for m in range(2):
                    nc.gpsimd.memset(t4[:, m, 0:1, :], 0.0)
                    nc.gpsimd.memset(t4[:, m, PH - 1 : PH, :], 0.0)
                    nc.gpsimd.memset(t4[:, m, :, 0:1], 0.0)
                    nc.gpsimd.memset(t4[:, m, :, PW - 1 : PW], 0.0)
                nc.gpsimd.memset(s8[P][:, :, NPAD : NPAD + 2], 0.0)
            # chunk 7 has dm=96: zero its tail partitions once
            nc.gpsimd.memset(s8[3][96:128, 1, :], 0.0)

            # E tiles in padded-grid layout (f32r), zeroed borders
            tpad = [
                tpadp.tile([128, NPAD], f32r, tag="tpad", name=f"tpad{c}")
                for c in range(8)
            ]
            for c in range(8):
                tf = tpad[c].bitcast(f32).rearrange(
                    "j (h w) -> j h w", h=PH, w=PW
                )
                nc.gpsimd.memset(tf[:, 0:1, :], 0.0)
                nc.gpsimd.memset(tf[:, PH - 1 : PH, :], 0.0)
                nc.gpsimd.memset(tf[:, :, 0:1], 0.0)
                nc.gpsimd.memset(tf[:, :, PW - 1 : PW], 0.0)

            # Patch-gather tiles (fp8): [j-part, pair-member, (p,q,c_out)].
            # Prefetched during the gram phase. Each gather dma_start costs
            # ~600ns on its issuing sequencer, so all 64 go on gpsimd (idle
            # after the memsets) to keep sync/scalar queues clear for the
            # exp/recip/cast chain.
            pt8 = {}
            for t, srcpad in enumerate((a8d, b8d)):
                for P in range(4):
                    pt = patp.tile(
                        [128, 2, KK], f8, tag="pat", name=f"pt{t}_{P}"
                    )
                    for m in range(2):
                        c = 2 * P + m
                        nhj = 4 if c < 7 else 3
                        for dh in range(nhj):
                            sap = bass.AP(
                                tensor=srcpad.ap().tensor,
                                offset=(4 * c + dh) * PW * C,
                                ap=[
                                    [C, Wp],
                                    [PW * C, 3],
                                    [C, 3],
                                    [1, C],
                                ],
                            )
                            nc.gpsimd.dma_start(
                                out=pt[32 * dh : 32 * (dh + 1), m, :],
                                in_=sap,
                            )
                    if P == 3:
                        # chunk 7 has only 3 dh rows: zero the tail rows
                        nc.gpsimd.memset(pt[96:128, 1, :], 0.0)
                    pt8[(t, P)] = pt

            with ExitStack() as ph1:
                psD = ph1.enter_context(
                    tc.tile_pool(name="psD", bufs=1, space="PSUM")
                )
                dpsall = psD.tile([1, L], f32, tag="dps", name="dpsall")
                dps = [dpsall[:, i0 : i0 + n] for (i0, n, _, _) in HALves]

                # Gram R = z.T@z per (j-chunk, i-chunk); E symmetric so only
                # i >= 128*jc is computed, rest mirrored by PE transpose.
                def ichunks(jc):
                    off = 128 * jc
                    out = []
                    while off < L:
                        n = min(512, L - off)
                        out.append((off, n))
                        off += n
                    return out

                with tc.tile_pool(name="psR", bufs=6, space="PSUM") as psR:
                    for g0, g1 in ((0, 3), (3, 6), (6, 8)):
                        grp = list(enumerate(JC))[g0:g1]
                        rps = {
                            c: [
                                psR.tile(
                                    [128, n], f32, tag="rps", name=f"rps{c}_{ci}"
                                )
                                for ci, (i0, n) in enumerate(ichunks(c))
                            ]
                            for c, _ in grp
                        }
                        # k-pair-major so early matmuls only need early z8
                        for k in range(9):
                            for c, (j0, dm) in grp:
                                for ci, (i0, n) in enumerate(ichunks(c)):
                                    nc.tensor.matmul(
                                        rps[c][ci][:dm, :],
                                        z8[k][:, :, j0 : j0 + dm],
                                        z8[k][:, :, i0 : i0 + n],
                                        start=(k == 0),
                                        stop=(k == 8),
                                        perf_mode=DR,
                                    )
                        for c, (j0, dm) in grp:
                            t3 = tpad[c].rearrange("j (h w) -> j h w", h=PH, w=PW)
                            for ci, (i0, n) in enumerate(ichunks(c)):
                                h0, nh = i0 // Wp, n // Wp
                                itv = t3[:dm, 1 + h0 : 1 + h0 + nh, 1 : 1 + Wp]
                                nc.vector.tensor_mul(
                                    itv,
                                    rps[c][ci][:dm, :],
                                    sb_inv_b[:dm, i0 : i0 + n],
                                )
                                nc.scalar.activation(
                                    itv,
                                    itv,
                                    mybir.ActivationFunctionType.Exp,
                                    scale=sb_inv_p[:dm, c : c + 1],
                                )

                # mirror lower-triangle blocks, then the softmax denominators
                with tc.tile_pool(name="psT", bufs=4, space="PSUM") as psT, \
                        tc.tile_pool(name="tbp", bufs=6) as tbp:
                    for c, (j0, dm) in enumerate(JC):
                        t3j = tpad[c].rearrange("j (h w) -> j h w", h=PH, w=PW)
                        nhj = dm // Wp
                        for ic in range(c):
                            t3s = tpad[ic].rearrange(
                                "j (h w) -> j h w", h=PH, w=PW
                            )
                            srcv = t3s[:128, 1 + 4 * c : 1 + 4 * c + nhj, 1 : 1 + Wp]
                            tbn = tbp.tile(
                                [128, 128], f32r, tag="tbn", name=f"tbn{c}_{ic}"
                            )
                            nc.vector.tensor_copy(tbn[:, :dm], srcv)
                            pst = psT.tile(
                                [128, 128], f32r, tag="pst", name=f"pst{c}_{ic}"
                            )
                            nc.tensor.transpose(pst[:dm, :128], tbn[:, :dm], idn[:, :])
                            nc.vector.tensor_copy(
                                t3j[:dm, 1 + 4 * ic : 1 + 4 * ic + 4, 1 : 1 + Wp],
                                pst[:dm, :128],
                            )
                        for hi, (i0, n, h0, nh) in enumerate(HALves):
                            nc.tensor.matmul(
                                dps[hi],
                                ones_k[:dm, :],
                                t3j[:dm, 1 + h0 : 1 + h0 + nh, 1 : 1 + Wp],
                                start=(c == 0),
                                stop=(c == 7),
                            )

                # 1/denom into SBUF; psD dies with ph1 before psB opens
                rtmp2 = const.tile([1, L], f32, tag="rtmp2")
                nc.vector.reciprocal_approx_fast(out=rtmp2[:, :], in_=dpsall[:, :])
                nc.vector.tensor_copy(recip_sb[:, :], rtmp2[:, :])

            # Broadcast (x DSC) across partitions via K=1 matmul; the recon
            # normalize reads it directly from PSUM (bank reserved through
            # the recon phase: 2 banks here + 6 psY banks = 8).
            psB = ctx.enter_context(
                tc.tile_pool(name="psB", bufs=1, space="PSUM")
            )
            bpsall = psB.tile([128, L], f32, tag="bps", name="bpsall")
            for hi, (i0, n, _, _) in enumerate(HALves):
                nc.tensor.matmul(
                    bpsall[:, i0 : i0 + n],
                    sc_m[:, :],
                    recip_sb[:, i0 : i0 + n],
                    start=True,
                    stop=True,
                )

            # Reconstruction of the deviation term, DoubleRow over pairs.
            # Outputs accumulate in grid-shaped PSUM tiles (rows of 34 incl
            # 2 junk cols) so each (p,q)-shifted rhs is a single contiguous
            # [j, 2, nh*34] stream (the DoubleRow-compatible 3D form). Two
            # sequential sweeps (tensor a then b), 6 PSUM banks each; sweep
            # a's output copy/DMA overlaps sweep b's matmuls.
            # Per chunk: normalize E by DSC*recip_i (DVE), then cast to fp8
            # with the uniform offset folded into the activation bias.
            SECS = [(0, 11), (11, 10), (21, 10)]  # (h0, nh) over l' rows
            with ExitStack() as ph2:
                psY = ph2.enter_context(
                    tc.tile_pool(name="psY", bufs=6, space="PSUM")
                )
                for t, dram in enumerate((ya_t, yb_t)):
                    yps = [
                        [
                            psY.tile(
                                [128, nh * PW],
                                f32,
                                tag="yps",
                                name=f"yps{t}_{cb}_{si}",
                            )
                            for si, (h0, nh) in enumerate(SECS)
                        ]
                        for cb in range(2)
                    ]
                    for P in range(4):
                        if t == 0:
                            s4 = s8[P][:, :, :NPAD].rearrange(
                                "j m (h w) -> j m h w", h=PH, w=PW
                            )
                            # top halves of both chunks first: the first
                            # recon section only reads grid rows <= 13
                            for hi, (i0, n, h0, nh) in enumerate(HALves):
                                for m in range(2):
                                    c = 2 * P + m
                                    j0, dm = JC[c]
                                    t3 = tpad[c].rearrange(
                                        "j (h w) -> j h w", h=PH, w=PW
                                    )
                                    itv = t3[
                                        :dm, 1 + h0 : 1 + h0 + nh, 1 : 1 + Wp
                                    ]
                                    nc.vector.tensor_mul(
                                        itv, itv, bpsall[:dm, i0 : i0 + n]
                                    )
                                    nc.scalar.activation(
                                        s4[:dm, m, 1 + h0 : 1 + h0 + nh, 1 : 1 + Wp],
                                        itv,
                                        mybir.ActivationFunctionType.Identity,
                                        bias=bias_u[:dm, :],
                                    )
                        for p in range(3):
                            for q in range(3):
                                for cb in range(2):
                                    lhs = pt8[(t, P)][
                                        :,
                                        :,
                                        (3 * p + q) * C
                                        + 128 * cb : (3 * p + q) * C
                                        + 128 * (cb + 1),
                                    ]
                                    for si, (h0, nh) in enumerate(SECS):
                                        g0 = (h0 - p + 2) * PW + (2 - q)
                                        nc.tensor.matmul(
                                            yps[cb][si][:, :],
                                            lhs,
                                            s8[P][:, :, g0 : g0 + nh * PW],
                                            start=(P == 0 and p == 0 and q == 0),
                                            stop=(P == 3 and p == 2 and q == 2),
                                            perf_mode=DR,
                                        )
                    for cb in range(2):
                        ysb = outp.tile(
                            [128, L], bf16, tag="ysb", name=f"ysb{t}_{cb}"
                        )
                        for si, (h0, nh) in enumerate(SECS):
                            ypv = yps[cb][si].rearrange(
                                "c (h w) -> c h w", h=nh, w=PW
                            )[:, :, 0:Wp]
                            ysv = ysb[:, h0 * Wp : (h0 + nh) * Wp].rearrange(
                                "c (h w) -> c h w", h=nh, w=Wp
                            )
                            if si % 2 == 0:
                                nc.vector.tensor_copy(ysv, ypv)
                            else:
                                nc.scalar.copy(ysv, ypv)
                        [nc.sync, nc.scalar, nc.sync, nc.scalar][
                            2 * t + cb
                        ].dma_start(
                            out=dram[128 * cb : 128 * (cb + 1), :], in_=ysb[:]
                        )

    nc.compile()
    return nc


def _get_program():
    if "nc" not in _CACHE:
        _CACHE["nc"] = _build_program()
    return _CACHE["nc"]


def _prep_core(A, B):
    """A, B: [31,32,256] float32 -> (input map, host uniform term [L, C])."""
    ap = np.zeros((PH, PW, C), np.float32)
    ap[1 : 1 + Hp, 1 : 1 + Wp] = A
    bp = np.zeros((PH, PW, C), np.float32)
    bp[1 : 1 + Hp, 1 : 1 + Wp] = B

    # patches [3,3,C,L] without materializing: strided windows
    def win(pad, p, q):
        return pad[p : p + Hp, q : q + Wp]  # [Hp, Wp, C]

    ss_a = np.zeros((Hp, Wp))
    ss_b = np.zeros((Hp, Wp))
    z8 = np.empty((1152, 2 * L), dtype=E4)
    zrow = np.empty((C, L), np.float32)
    for p in range(3):
        for q in range(3):
            wa = win(ap, p, q).astype(np.float64)
            wb = win(bp, p, q).astype(np.float64)
            ss_a += (wa * wa).sum(-1)
            ss_b += (wb * wb).sum(-1)
            np.multiply(
                win(ap, p, q).reshape(L, C).T,
                win(bp, p, q).reshape(L, C).T,
                out=zrow,
            )
            kk = 2 * (3 * p + q)  # two 128-row slices per (p,q)
            for half in range(2):
                rows = zrow[128 * half : 128 * (half + 1)]
                pair, mm = divmod(kk + half, 2)
                z8[128 * pair : 128 * (pair + 1), mm * L : (mm + 1) * L] = (
                    ZSC * rows
                ).astype(E4)
    inv = (
        1.0
        / np.maximum(np.sqrt(ss_a), 1e-4)
        / np.maximum(np.sqrt(ss_b), 1e-4)
    ).reshape(-1)

    # host uniform term: y_mean[l', c] = u * sum_pq mask * window-sum
    u = 1.0 / L
    Ug = np.zeros((PH, PW))
    Ug[1 : 1 + Hp, 1 : 1 + Wp] = 1.0
    ymean_a = np.zeros((L, C))
    ymean_b = np.zeros((L, C))
    for p in range(3):
        for q in range(3):
            w = Ug[2 - p : 2 - p + Hp, 2 - q : 2 - q + Wp].reshape(L, 1)
            ymean_a += u * w @ win(ap, p, q).astype(np.float64).sum((0, 1))[None, :]
            ymean_b += u * w @ win(bp, p, q).astype(np.float64).sum((0, 1))[None, :]

    inp = {
        "z8": z8,
        "a8": (PSC * ap).astype(E4),
        "b8": (PSC * bp).astype(E4),
        "inv_p": np.ascontiguousarray(
            np.pad(10.0 * inv, (0, 1024 - L)).reshape(8, 128).T.astype(np.float32)
        ),
        "inv_f": (inv / (ZSC * ZSC)).reshape(1, L).astype(np.float32),
    }
    return inp, ymean_a, ymean_b


def _assemble(res, ymean_a, ymean_b):
    """Device bf16 deviation outputs [C, L] -> full [Hp, Wp, C] pair."""
    sc = 1.0 / (DSC * PSC)
    ya = ymean_a + sc * res["ya_t"].astype(np.float64).T
    yb = ymean_b + sc * res["yb_t"].astype(np.float64).T
    return (
        ya.reshape(Hp, Wp, C).astype(np.float32),
        yb.reshape(Hp, Wp, C).astype(np.float32),
    )


def kernel(x, mask):
    x = np.asarray(x, dtype=np.float32)
    in_maps = []
    hosts = []
    for b in range(B_IMG):
        xb = x[b]
        im, ha, hb = _prep_core(xb[:-1], xb[1:])
        in_maps.append(im)
        hosts.append((ha, hb))
        xt = np.ascontiguousarray(xb.transpose(1, 0, 2))
        im, ha, hb = _prep_core(xt[1:], xt[:-1])
        in_maps.append(im)
        hosts.append((ha, hb))

    from concourse.bass_utils import run_bass_kernel_spmd

    nc = _get_program()
    res = run_bass_kernel_spmd(nc, in_maps, list(range(8))).results

    out = np.empty((B_IMG, H_IMG, W_IMG, C), np.float32)
    for b in range(B_IMG):
        yl, yr = _assemble(res[2 * b], *hosts[2 * b])
        ylr = np.concatenate(
            [yr[:1], (yr[1:] + yl[:-1]) * 0.5, yl[-1:]], axis=0
        )
        yt, yb = _assemble(res[2 * b + 1], *hosts[2 * b + 1])
        yt = yt.transpose(1, 0, 2)
        yb = yb.transpose(1, 0, 2)
        ytb = np.concatenate(
            [yt[:, :1], (yt[:, 1:] + yb[:, :-1]) * 0.5, yb[:, -1:]], axis=1
        )
        out[b] = (ylr + ytb) * 0.5
    return out


# revision 18
# speedup vs baseline: 1.8370x; 1.0392x over previous
"""EnvironmentConsistentAttention on 8 trn2 cores — fp8 DoubleRow version.

Sharding: 4 images x 2 directions (vertical/horizontal neighbor pairs) = 8
independent units, one per core. The horizontal direction of image x equals
the vertical direction of x spatially transposed, so a single SPMD program
handles both: given shifted maps A, B [31,32,256] it returns the fp8
*deviation* reconstruction; the exact uniform-attention part is added on the
host.

Math per core (Hp=31, Wp=32, C=256, L=992, k=3):
  pa[(p,q,c), l] = A_pad[h+p, w+q, c];  z = pa*pb  [2304, L]
  R = z.T @ z;  att = 10*inv_i*inv_j*R;  S = softmax(att, axis=j)
  y = conv_transpose(S, pa)  (and pb)

Key numeric fact for this problem: S is extremely close to uniform (u=1/L),
and y is dominated by the rank-1 uniform term. So split S = u + D and
compute only the deviation term on the accelerator with fp8e4m3 DoubleRow
matmuls (K=256 per pass at 0.5 cycles/row = 4x f32r throughput):
  - gram: z quantized to fp8 on host (scale 4), 9 partition-pair matmul
    groups; exp/softmax-denominator stay f32 (as in the f32r kernel:
    symmetric upper-triangle + PE-transpose mirror; ones-matmul colsums).
  - recon: D = (recip_i*E_ij - u) scaled by 1024, cast to fp8 on the ACT
    engine (Identity activation with bias=-1024/L), patches fp8 from host
    (scale 16). DoubleRow over 4 j-chunk-pairs.
  - uniform term: y_mean[l',c] = u * sum_pq mask(l',p,q) * window_sum(pad)
    computed exactly on host and added back; device output is bf16 (it only
    carries the small deviation term).
Measured model error of this scheme vs the jax reference: l2 ~4.4e-5.
"""

import numpy as np
import ml_dtypes

Hp, Wp, C = 31, 32, 256
L = Hp * Wp            # 992
PH, PW = Hp + 2, Wp + 2  # 33, 34 padded grid
NPAD = PH * PW         # 1122
KK = 9 * C             # 2304
JC = [(128 * c, 128 if c < 7 else 96) for c in range(8)]   # j/l chunks
HALves = [(0, 512, 0, 16), (512, 480, 16, 15)]  # (i0, n, h0, nh) over i/l'
B_IMG, H_IMG, W_IMG = 4, 32, 32

ZSC = 4.0       # host z fp8 scale (per factor; gram R is scaled by ZSC^2)
PSC = 16.0      # host patch fp8 scale
DSC = 1024.0    # device D fp8 scale
E4 = ml_dtypes.float8_e4m3

_CACHE = {}


def _build_program():
    import concourse.bass as bass
    import concourse.tile as tile
    from concourse import bacc, mybir

    f32 = mybir.dt.float32
    f32r = mybir.dt.float32r
    f8 = mybir.dt.float8e4
    bf16 = mybir.dt.bfloat16
    DR = mybir.MatmulPerfMode.DoubleRow

    nc = bacc.Bacc("TRN2", target_bir_lowering=False, debug=False)

    z8d = nc.dram_tensor("z8", [1152, 2 * L], f8, kind="ExternalInput")
    a8d = nc.dram_tensor("a8", [PH, PW, C], f8, kind="ExternalInput")
    b8d = nc.dram_tensor("b8", [PH, PW, C], f8, kind="ExternalInput")
    inv_p = nc.dram_tensor("inv_p", [128, 8], f32, kind="ExternalInput")
    inv_f = nc.dram_tensor("inv_f", [1, L], f32, kind="ExternalInput")
    ya_t = nc.dram_tensor("ya_t", [C, L], bf16, kind="ExternalOutput")
    yb_t = nc.dram_tensor("yb_t", [C, L], bf16, kind="ExternalOutput")

    with tile.TileContext(nc) as tc:
        from contextlib import ExitStack

        with ExitStack() as ctx:
            const = ctx.enter_context(tc.tile_pool(name="const", bufs=1))
            outp = ctx.enter_context(tc.tile_pool(name="outp", bufs=4))
            tpadp = ctx.enter_context(tc.tile_pool(name="tpad", bufs=8))
            z8p = ctx.enter_context(tc.tile_pool(name="z8p", bufs=9))
            patp = ctx.enter_context(tc.tile_pool(name="pat", bufs=8))
            s8p = ctx.enter_context(tc.tile_pool(name="s8p", bufs=4))

            # gpsimd dma_start goes through a slow DIRECT2D path and clogs
            # the gpsimd sequencer (whose memsets gate the exp/cast chain):
            # only sync and scalar issue DMAs.
            dma_engs = [nc.sync, nc.scalar]

            # z8 pair tiles first: the gram chases these
            z8 = []
            for k in range(9):
                zt = z8p.tile([128, 2, L], f8, tag="z8", name=f"z8_{k}")
                dma_engs[k % 2].dma_start(
                    out=zt[:], in_=z8d[128 * k : 128 * (k + 1), :]
                )
                z8.append(zt)

            # Constants
            sb_inv_p = const.tile([128, 8], f32, tag="invp")
            sb_inv_b = const.tile([128, L], f32, tag="invb")
            nc.sync.dma_start(out=sb_inv_p[:], in_=inv_p[:, :])
            nc.sync.dma_start(
                out=sb_inv_b[:], in_=inv_f.ap().to_broadcast([128, L])
            )
            ones_f = const.tile([128, 128], f32, tag="onesf")
            nc.vector.memset(ones_f[:], 1.0)
            ones_k = const.tile([128, 1], f32r, tag="onesk")
            nc.scalar.copy(ones_k[:], ones_f[:, 0:1])
            sc_f = const.tile([1, 128], f32, tag="scf")
            nc.vector.memset(sc_f[:], DSC)
            sc_m = const.tile([1, 128], f32r, tag="scm")
            nc.scalar.copy(sc_m[:], sc_f[:, :])
            bias_u = const.tile([128, 1], f32, tag="biasu")
            nc.vector.memset(bias_u[:], -DSC / L)
            from concourse.masks import make_identity

            idn_f = const.tile([128, 128], f32, tag="idnf")
            idn = const.tile([128, 128], f32r, tag="idn")
            make_identity(nc, idn_f[:])
            nc.scalar.copy(idn[:], idn_f[:])
            recip_sb = const.tile([1, L], f32r, tag="recip")

            # S.T deviation tiles (fp8, padded grid + 2 overrun cols) per
            # j-chunk pair. Recon reads them as flat [j, m, nh*34] streams.
            s8 = [
                s8p.tile([128, 2, NPAD + 2], f8, tag="s8", name=f"s8_{P}")
                for P in range(4)
            ]
            for P in range(4):
                t4 = s8[P][:, :, :NPAD].rearrange(
                    "j m (h w) -> j m h w", h=PH, w=PW
                )
                for m in range(2):
                    nc.gpsimd.memset(t4[:, m, 0:1, :], 0.0)
                    nc.gpsimd.memset(t4[:, m, PH - 1 : PH, :], 0.0)
                    nc.gpsimd.memset(t4[:, m, :, 0:1], 0.0)
                    nc.gpsimd.memset(t4[:, m, :, PW - 1 : PW], 0.0)
                nc.gpsimd.memset(s8[P][:, :, NPAD : NPAD + 2], 0.0)
            # chunk 7 has dm=96: zero its tail partitions once
            nc.gpsimd.memset(s8[3][96:128, 1, :], 0.0)

            # E tiles in padded-grid layout (f32r), zeroed borders
            tpad = [
                tpadp.tile([128, NPAD], f32r, tag="tpad", name=f"tpad{c}")
                for c in range(8)
            ]
            for c in range(8):
                tf = tpad[c].bitcast(f32).rearrange(
                    "j (h w) -> j h w", h=PH, w=PW
                )
                nc.gpsimd.memset(tf[:, 0:1, :], 0.0)
                nc.gpsimd.memset(tf[:, PH - 1 : PH, :], 0.0)
                nc.gpsimd.memset(tf[:, :, 0:1], 0.0)
                nc.gpsimd.memset(tf[:, :, PW - 1 : PW], 0.0)

            # Patch-gather tiles (fp8): [j-part, pair-member, (p,q,c_out)].
            # Prefetched during the gram phase. Each gather dma_start costs
            # ~600ns on its issuing sequencer, so all 64 go on gpsimd (idle
            # after the memsets) to keep sync/scalar queues clear for the
            # exp/recip/cast chain.
            pt8 = {}
            for t, srcpad in enumerate((a8d, b8d)):
                for P in range(4):
                    pt = patp.tile(
                        [128, 2, KK], f8, tag="pat", name=f"pt{t}_{P}"
                    )
                    for m in range(2):
                        c = 2 * P + m
                        nhj = 4 if c < 7 else 3
                        for dh in range(nhj):
                            sap = bass.AP(
                                tensor=srcpad.ap().tensor,
                                offset=(4 * c + dh) * PW * C,
                                ap=[
                                    [C, Wp],
                                    [PW * C, 3],
                                    [C, 3],
                                    [1, C],
                                ],
                            )
                            nc.gpsimd.dma_start(
                                out=pt[32 * dh : 32 * (dh + 1), m, :],
                                in_=sap,
                            )
                    if P == 3:
                        # chunk 7 has only 3 dh rows: zero the tail rows
                        nc.gpsimd.memset(pt[96:128, 1, :], 0.0)
                    pt8[(t, P)] = pt

            with ExitStack() as ph1:
                psD = ph1.enter_context(
                    tc.tile_pool(name="psD", bufs=1, space="PSUM")
                )
                dpsall = psD.tile([1, L], f32, tag="dps", name="dpsall")
                dps = [dpsall[:, i0 : i0 + n] for (i0, n, _, _) in HALves]

                # Gram R = z.T@z per (j-chunk, i-chunk); E symmetric so only
                # i >= 128*jc is computed, rest mirrored by PE transpose.
                def ichunks(jc):
                    off = 128 * jc
                    out = []
                    while off < L:
                        n = min(512, L - off)
                        out.append((off, n))
                        off += n
                    return out

                with tc.tile_pool(name="psR", bufs=6, space="PSUM") as psR:
                    for g0, g1 in ((0, 3), (3, 6), (6, 8)):
                        grp = list(enumerate(JC))[g0:g1]
                        rps = {
                            c: [
                                psR.tile(
                                    [128, n], f32, tag="rps", name=f"rps{c}_{ci}"
                                )
                                for ci, (i0, n) in enumerate(ichunks(c))
                            ]
                            for c, _ in grp
                        }
                        # k-pair-major so early matmuls only need early z8
                        for k in range(9):
                            for c, (j0, dm) in grp:
                                for ci, (i0, n) in enumerate(ichunks(c)):
                                    nc.tensor.matmul(
                                        rps[c][ci][:dm, :],
                                        z8[k][:, :, j0 : j0 + dm],
                                        z8[k][:, :, i0 : i0 + n],
                                        start=(k == 0),
                                        stop=(k == 8),
                                        perf_mode=DR,
                                    )
                        for c, (j0, dm) in grp:
                            t3 = tpad[c].rearrange("j (h w) -> j h w", h=PH, w=PW)
                            for ci, (i0, n) in enumerate(ichunks(c)):
                                h0, nh = i0 // Wp, n // Wp
                                itv = t3[:dm, 1 + h0 : 1 + h0 + nh, 1 : 1 + Wp]
                                nc.vector.tensor_mul(
                                    itv,
                                    rps[c][ci][:dm, :],
                                    sb_inv_b[:dm, i0 : i0 + n],
                                )
                                nc.scalar.activation(
                                    itv,
                                    itv,
                                    mybir.ActivationFunctionType.Exp,
                                    scale=sb_inv_p[:dm, c : c + 1],
                                )

                # mirror lower-triangle blocks, then the softmax denominators
                with tc.tile_pool(name="psT", bufs=4, space="PSUM") as psT, \
                        tc.tile_pool(name="tbp", bufs=6) as tbp:
                    for c, (j0, dm) in enumerate(JC):
                        t3j = tpad[c].rearrange("j (h w) -> j h w", h=PH, w=PW)
                        nhj = dm // Wp
                        for ic in range(c):
                            t3s = tpad[ic].rearrange(
                                "j (h w) -> j h w", h=PH, w=PW
                            )
                            srcv = t3s[:128, 1 + 4 * c : 1 + 4 * c + nhj, 1 : 1 + Wp]
                            tbn = tbp.tile(
                                [128, 128], f32r, tag="tbn", name=f"tbn{c}_{ic}"
                            )
                            nc.vector.tensor_copy(tbn[:, :dm], srcv)
                            pst = psT.tile(
                                [128, 128], f32r, tag="pst", name=f"pst{c}_{ic}"
                            )
                            nc.tensor.transpose(pst[:dm, :128], tbn[:, :dm], idn[:, :])
                            nc.scalar.copy(
                                t3j[:dm, 1 + 4 * ic : 1 + 4 * ic + 4, 1 : 1 + Wp],
                                pst[:dm, :128],
                            )
                        for hi, (i0, n, h0, nh) in enumerate(HALves):
                            nc.tensor.matmul(
                                dps[hi],
                                ones_k[:dm, :],
                                t3j[:dm, 1 + h0 : 1 + h0 + nh, 1 : 1 + Wp],
                                start=(c == 0),
                                stop=(c == 7),
                            )

                # 1/denom into SBUF; psD dies with ph1 before psB opens
                rtmp2 = const.tile([1, L], f32, tag="rtmp2")
                nc.vector.reciprocal_approx_fast(out=rtmp2[:, :], in_=dpsall[:, :])
                nc.vector.tensor_copy(recip_sb[:, :], rtmp2[:, :])

            # Broadcast (x DSC) across partitions via K=1 matmul; the recon
            # normalize reads it directly from PSUM (bank reserved through
            # the recon phase: 2 banks here + 6 psY banks = 8).
            psB = ctx.enter_context(
                tc.tile_pool(name="psB", bufs=1, space="PSUM")
            )
            bpsall = psB.tile([128, L], f32, tag="bps", name="bpsall")
            for hi, (i0, n, _, _) in enumerate(HALves):
                nc.tensor.matmul(
                    bpsall[:, i0 : i0 + n],
                    sc_m[:, :],
                    recip_sb[:, i0 : i0 + n],
                    start=True,
                    stop=True,
                )

            # Reconstruction of the deviation term, DoubleRow over pairs.
            # Outputs accumulate in grid-shaped PSUM tiles (rows of 34 incl
            # 2 junk cols) so each (p,q)-shifted rhs is a single contiguous
            # [j, 2, nh*34] stream (the DoubleRow-compatible 3D form). Two
            # sequential sweeps (tensor a then b), 6 PSUM banks each; sweep
            # a's output copy/DMA overlaps sweep b's matmuls.
            # Per chunk: normalize E by DSC*recip_i (DVE), then cast to fp8
            # with the uniform offset folded into the activation bias.
            SECS = [(0, 11), (11, 10), (21, 10)]  # (h0, nh) over l' rows
            with ExitStack() as ph2:
                psY = ph2.enter_context(
                    tc.tile_pool(name="psY", bufs=6, space="PSUM")
                )
                for t, dram in enumerate((ya_t, yb_t)):
                    yps = [
                        [
                            psY.tile(
                                [128, nh * PW],
                                f32,
                                tag="yps",
                                name=f"yps{t}_{cb}_{si}",
                            )
                            for si, (h0, nh) in enumerate(SECS)
                        ]
                        for cb in range(2)
                    ]
                    for P in range(4):
                        if t == 0:
                            s4 = s8[P][:, :, :NPAD].rearrange(
                                "j m (h w) -> j m h w", h=PH, w=PW
                            )
                            # top halves of both chunks first: the first
                            # recon section only reads grid rows <= 13
                            for hi, (i0, n, h0, nh) in enumerate(HALves):
                                for m in range(2):
                                    c = 2 * P + m
                                    j0, dm = JC[c]
                                    t3 = tpad[c].rearrange(
                                        "j (h w) -> j h w", h=PH, w=PW
                                    )
                                    itv = t3[
                                        :dm, 1 + h0 : 1 + h0 + nh, 1 : 1 + Wp
                                    ]
                                    nc.vector.tensor_mul(
                                        itv, itv, bpsall[:dm, i0 : i0 + n]
                                    )
                                    nc.scalar.activation(
                                        s4[:dm, m, 1 + h0 : 1 + h0 + nh, 1 : 1 + Wp],
                                        itv,
                                        mybir.ActivationFunctionType.Identity,
                                        bias=bias_u[:dm, :],
                                    )
                        for p in range(3):
                            for q in range(3):
                                for cb in range(2):
                                    lhs = pt8[(t, P)][
                                        :,
                                        :,
                                        (3 * p + q) * C
                                        + 128 * cb : (3 * p + q) * C
                                        + 128 * (cb + 1),
                                    ]
                                    for si, (h0, nh) in enumerate(SECS):
                                        g0 = (h0 - p + 2) * PW + (2 - q)
                                        nc.tensor.matmul(
                                            yps[cb][si][:, :],
                                            lhs,
                                            s8[P][:, :, g0 : g0 + nh * PW],
                                            start=(P == 0 and p == 0 and q == 0),
                                            stop=(P == 3 and p == 2 and q == 2),
                                            perf_mode=DR,
                                        )
                    for cb in range(2):
                        ysb = outp.tile(
                            [128, L], bf16, tag="ysb", name=f"ysb{t}_{cb}"
                        )
                        for si, (h0, nh) in enumerate(SECS):
                            ypv = yps[cb][si].rearrange(
                                "c (h w) -> c h w", h=nh, w=PW
                            )[:, :, 0:Wp]
                            ysv = ysb[:, h0 * Wp : (h0 + nh) * Wp].rearrange(
                                "c (h w) -> c h w", h=nh, w=Wp
                            )
                            if si % 2 == 0:
                                nc.vector.tensor_copy(ysv, ypv)
                            else:
                                nc.scalar.copy(ysv, ypv)
                        [nc.sync, nc.scalar, nc.sync, nc.scalar][
                            2 * t + cb
                        ].dma_start(
                            out=dram[128 * cb : 128 * (cb + 1), :], in_=ysb[:]
                        )

    nc.compile()
    return nc


def _get_program():
    if "nc" not in _CACHE:
        _CACHE["nc"] = _build_program()
    return _CACHE["nc"]


def _prep_core(A, B):
    """A, B: [31,32,256] float32 -> (input map, host uniform term [L, C])."""
    ap = np.zeros((PH, PW, C), np.float32)
    ap[1 : 1 + Hp, 1 : 1 + Wp] = A
    bp = np.zeros((PH, PW, C), np.float32)
    bp[1 : 1 + Hp, 1 : 1 + Wp] = B

    # patches [3,3,C,L] without materializing: strided windows
    def win(pad, p, q):
        return pad[p : p + Hp, q : q + Wp]  # [Hp, Wp, C]

    ss_a = np.zeros((Hp, Wp))
    ss_b = np.zeros((Hp, Wp))
    z8 = np.empty((1152, 2 * L), dtype=E4)
    zrow = np.empty((C, L), np.float32)
    for p in range(3):
        for q in range(3):
            wa = win(ap, p, q).astype(np.float64)
            wb = win(bp, p, q).astype(np.float64)
            ss_a += (wa * wa).sum(-1)
            ss_b += (wb * wb).sum(-1)
            np.multiply(
                win(ap, p, q).reshape(L, C).T,
                win(bp, p, q).reshape(L, C).T,
                out=zrow,
            )
            kk = 2 * (3 * p + q)  # two 128-row slices per (p,q)
            for half in range(2):
                rows = zrow[128 * half : 128 * (half + 1)]
                pair, mm = divmod(kk + half, 2)
                z8[128 * pair : 128 * (pair + 1), mm * L : (mm + 1) * L] = (
                    ZSC * rows
                ).astype(E4)
    inv = (
        1.0
        / np.maximum(np.sqrt(ss_a), 1e-4)
        / np.maximum(np.sqrt(ss_b), 1e-4)
    ).reshape(-1)

    # host uniform term: y_mean[l', c] = u * sum_pq mask * window-sum
    u = 1.0 / L
    Ug = np.zeros((PH, PW))
    Ug[1 : 1 + Hp, 1 : 1 + Wp] = 1.0
    ymean_a = np.zeros((L, C))
    ymean_b = np.zeros((L, C))
    for p in range(3):
        for q in range(3):
            w = Ug[2 - p : 2 - p + Hp, 2 - q : 2 - q + Wp].reshape(L, 1)
            ymean_a += u * w @ win(ap, p, q).astype(np.float64).sum((0, 1))[None, :]
            ymean_b += u * w @ win(bp, p, q).astype(np.float64).sum((0, 1))[None, :]

    inp = {
        "z8": z8,
        "a8": (PSC * ap).astype(E4),
        "b8": (PSC * bp).astype(E4),
        "inv_p": np.ascontiguousarray(
            np.pad(10.0 * inv, (0, 1024 - L)).reshape(8, 128).T.astype(np.float32)
        ),
        "inv_f": (inv / (ZSC * ZSC)).reshape(1, L).astype(np.float32),
    }
    return inp, ymean_a, ymean_b


def _assemble(res, ymean_a, ymean_b):
    """Device bf16 deviation outputs [C, L] -> full [Hp, Wp, C] pair."""
    sc = 1.0 / (DSC * PSC)
    ya = ymean_a + sc * res["ya_t"].astype(np.float64).T
    yb = ymean_b + sc * res["yb_t"].astype(np.float64).T
    return (
        ya.reshape(Hp, Wp, C).astype(np.float32),
        yb.reshape(Hp, Wp, C).astype(np.float32),
    )


def kernel(x, mask):
    x = np.asarray(x, dtype=np.float32)
    in_maps = []
    hosts = []
    for b in range(B_IMG):
        xb = x[b]
        im, ha, hb = _prep_core(xb[:-1], xb[1:])
        in_maps.append(im)
        hosts.append((ha, hb))
        xt = np.ascontiguousarray(xb.transpose(1, 0, 2))
        im, ha, hb = _prep_core(xt[1:], xt[:-1])
        in_maps.append(im)
        hosts.append((ha, hb))

    from concourse.bass_utils import run_bass_kernel_spmd

    nc = _get_program()
    res = run_bass_kernel_spmd(nc, in_maps, list(range(8))).results

    out = np.empty((B_IMG, H_IMG, W_IMG, C), np.float32)
    for b in range(B_IMG):
        yl, yr = _assemble(res[2 * b], *hosts[2 * b])
        ylr = np.concatenate(
            [yr[:1], (yr[1:] + yl[:-1]) * 0.5, yl[-1:]], axis=0
        )
        yt, yb = _assemble(res[2 * b + 1], *hosts[2 * b + 1])
        yt = yt.transpose(1, 0, 2)
        yb = yb.transpose(1, 0, 2)
        ytb = np.concatenate(
            [yt[:, :1], (yt[:, 1:] + yb[:, :-1]) * 0.5, yb[:, -1:]], axis=1
        )
        out[b] = (ylr + ytb) * 0.5
    return out
